# revision 1
# baseline (speedup 1.0000x reference)
"""Trainium2 Bass kernel for nn_CrossResonanceLayer (sparse_attention).

Math (reference):
  w  = softmax(phase_weights)                          (L,)
  B_aligned = circconv(B, w)          = C1 @ B[b]      C1[l,m] = w[(l-m)%L]
  fire = gate(A)  -> scalar flag (host, tiny BxB math on pooled vectors)
  windowed local attention (radius 4) on (A, B_aligned), layernorm(A + rel)
  A_out = flag ? normed : A
  B_out = circconv(A_out, roll(w[::-1],1)) = C1^T @ A_out[b]

Sharding: 8 cores = (batch b in 0..3) x (sequence half h in 0..1).
Each core runs conv1 (own half rows + 128-row halo), attention + LN for its
half, and a *partial* conv2 (contribution of its own A_out rows to the FULL
B_out of its batch). Host sums the two partials per batch -> no cross-core
communication, no collectives.

Attention runs in feature-major layout (d on partitions, l on the free dim)
so the +-4 windowed shifts are plain free-dim slice offsets (no data
movement). Reductions/broadcasts over d use ones-matmuls on the PE.

The score projections are folded: scores = (A Wq^T)(B_al Wk^T)^T/sqrt(d)
= A (Wq^T Wk / sqrt(d)) B_al^T, so Wk never multiplies B_al on device.

Precision: conv1 / attention in bf16 (error lands ~1e-4 absolute, far below
signal), LN + residual in fp32, conv2 in fp32r (tf32-like, ~1.5e-4 rel).
"""
import sys

sys.path.insert(0, "/opt/trn_rl_repo")

from contextlib import ExitStack

import numpy as np
import ml_dtypes

import concourse.bass as bass
import concourse.tile as tile
from concourse import mybir
from concourse.bass_utils import run_bass_kernel_spmd
from concourse.masks import make_identity

F32 = mybir.dt.float32
F32R = mybir.dt.float32r
BF16 = mybir.dt.bfloat16
AOP = mybir.AluOpType
ACTF = mybir.ActivationFunctionType

Bsz, L, D = 4, 4096, 512
HALF = L // 2              # 2048 rows per core
HALO = 8                   # windowed attention needs only +-4
WID = HALF + 2 * HALO      # 2064 halo-extended rows
NT = HALF // 128           # 16 own l-tiles
KT = L // 128              # 32 k-tiles along L
DT = D // 128              # 4 d-tiles
RADIUS = 4
LN_EPS = 1e-5
THRESHOLD = 0.15


def _split_excess_waits(nc, max_waits=1):
    """This walrus build accepts at most one sem-wait command per instruction.
    Move excess waits onto same-engine NOPs placed right before the owner."""
    ctr = 0
    for fn in nc.m.functions:
        for bb in fn.blocks:
            out = []
            changed = False
            for inst in bb.instructions:
                si = inst.sync_info
                if si is not None and len(si.on_wait) > max_waits:
                    waits = list(si.on_wait)
                    keep = waits[-max_waits:]
                    extra = waits[:-max_waits]
                    for i in range(0, len(extra), max_waits):
                        nop = mybir.InstNoOp(name=f"waitsplit-{ctr}")
                        ctr += 1
                        nop.engine = inst.engine
                        nop.sync_info = mybir.SyncInfo(
                            on_wait=extra[i : i + max_waits], on_update=[]
                        )
                        out.append(nop)
                    si.on_wait = keep
                    changed = True
                out.append(inst)
            if changed:
                bb.instructions = out
    return ctr


def _build_nc():
    nc = bass.Bass("TRN2", target_bir_lowering=False, debug=False, num_devices=8)

    # ---- inputs (per core) ----
    Bin = nc.dram_tensor("Bin", [L, D], BF16, kind="ExternalInput").ap()
    # CT1v[l, j] = C1[(own0-HALO+j)%L, l]  (transposed slice of the circulant)
    CT1v = nc.dram_tensor("CT1v", [L, WID], BF16, kind="ExternalInput").ap()
    C2 = nc.dram_tensor("C2", [HALF, L], F32R, kind="ExternalInput").ap()
    AT16 = nc.dram_tensor("AT16", [D, HALF], BF16, kind="ExternalInput").ap()
    Apb = nc.dram_tensor("Apb", [HALF, D], F32, kind="ExternalInput").ap()  # A + bo
    Wqk = nc.dram_tensor("Wqk", [D, D], BF16, kind="ExternalInput").ap()   # WqT@Wk/sqrt(d)
    WvT = nc.dram_tensor("WvT", [D, D], BF16, kind="ExternalInput").ap()
    WoT = nc.dram_tensor("WoT", [D, D], BF16, kind="ExternalInput").ap()
    gam = nc.dram_tensor("gam", [D], F32, kind="ExternalInput").ap()   # flag*ln_scale
    bet2 = nc.dram_tensor("bet2", [D], F32, kind="ExternalInput").ap() # flag*ln_bias-(1-flag)*bo
    flagc = nc.dram_tensor("flagc", [1], F32, kind="ExternalInput").ap()  # 1-flag

    # ---- outputs ----
    A_out = nc.dram_tensor("A_out", [HALF, D], F32, kind="ExternalOutput").ap()
    BT_part = nc.dram_tensor("BT_part", [D, L], F32, kind="ExternalOutput").ap()

    def bcast(row_ap, parts=128):
        return bass.AP(
            tensor=row_ap.tensor,
            offset=row_ap.offset,
            ap=[[0, parts]] + list(row_ap.ap),
        )

    ts = bass.ts
    offs = [i - RADIUS for i in range(9)]
    # conv1/VT free-dim chunks (512 wide -> 4 PSUM banks live)
    C1CH = [(c, min(D, WID - c)) for c in range(0, WID, D)]

    with tile.TileContext(nc) as tc, ExitStack() as ctx:
        consts = ctx.enter_context(tc.tile_pool(name="consts", bufs=1))
        gamB = consts.tile([128, D], F32)
        nc.sync.dma_start(gamB[:], bcast(gam))
        bet2B = consts.tile([128, D], F32)
        nc.sync.dma_start(bet2B[:], bcast(bet2))
        flagcS = consts.tile([128, 1], F32)
        nc.sync.dma_start(flagcS[:], bcast(flagc))
        epsS = consts.tile([128, 1], F32)
        nc.vector.memset(epsS[:], LN_EPS)
        ones128 = consts.tile([128, 1], BF16)
        nc.vector.memset(ones128[:], 1.0)
        ones1 = consts.tile([1, 128], BF16)
        nc.vector.memset(ones1[:], 1.0)
        onesPS = consts.tile([128, 1], F32)
        nc.vector.memset(onesPS[:], 1.0)

        wpool = ctx.enter_context(tc.tile_pool(name="wpool", bufs=1))
        wqkAll = wpool.tile([128, DT, D], BF16)
        nc.sync.dma_start(wqkAll[:], Wqk.rearrange("(kd p) d -> p kd d", p=128))
        wvtAll = wpool.tile([128, DT, D], BF16)
        nc.sync.dma_start(wvtAll[:], WvT.rearrange("(kd p) d -> p kd d", p=128))
        wotAll = wpool.tile([128, DT, D], BF16)
        nc.sync.dma_start(wotAll[:], WoT.rearrange("(kd p) d -> p kd d", p=128))

        # persistent feature-major activations
        persist = ctx.enter_context(tc.tile_pool(name="persist", bufs=1))
        balt = [persist.tile([128, WID], BF16, tag=f"bt{k}", name=f"bt{k}") for k in range(DT)]
        vtt = [persist.tile([128, WID], BF16, tag=f"vt{k}", name=f"vt{k}") for k in range(DT)]
        ptt = [persist.tile([128, HALF], BF16, tag=f"pt{k}", name=f"pt{k}") for k in range(DT)]
        ctxu = [persist.tile([128, HALF], BF16, tag=f"cu{k}", name=f"cu{k}") for k in range(DT)]
        aout = [persist.tile([128, D], F32R, tag=f"ao{t}", name=f"ao{t}") for t in range(NT)]

        # small row-tiles + LN pools (live through the whole interleave)
        p3a = ctx.enter_context(tc.tile_pool(name="p3a", bufs=1))
        p3c = ctx.enter_context(tc.tile_pool(name="p3c", bufs=2))

        # ---------------- emitters ----------------
        def emit_conv1_chunk(ct1p, ps1, bsbAll, ct1r, c0, cw):
            pss = [ps1.tile([128, D], F32, tag=f"ps{m}", name=f"ps{m}_{c0}")
                   for m in range(DT)]
            for kg in range(KT // 4):
                ct1g = ct1p.tile([128, 4, D], BF16, tag="ct1g")
                nc.sync.dma_start(ct1g[:, :, 0:cw], ct1r[kg, :, :, c0 : c0 + cw])
                for kk in range(4):
                    k = kg * 4 + kk
                    for m in range(DT):
                        nc.tensor.matmul(
                            pss[m][:, 0:cw],
                            bsbAll[:, k, ts(m, 128)],
                            ct1g[:, kk, 0:cw],
                            start=(k == 0), stop=(k == KT - 1),
                        )
            for m in range(DT):
                nc.scalar.copy(balt[m][:, c0 : c0 + cw], pss[m][:, 0:cw])

        def emit_vt_chunk(psV, c0, cw):
            for m in range(DT):
                ps = psV.tile([128, D], F32, tag="psv")
                for kd in range(DT):
                    nc.tensor.matmul(
                        ps[:, 0:cw],
                        wvtAll[:, kd, ts(m, 128)],
                        balt[kd][:, c0 : c0 + cw],
                        start=(kd == 0), stop=(kd == DT - 1),
                    )
                nc.scalar.copy(vtt[m][:, c0 : c0 + cw], ps[:, 0:cw])

        def emit_attn_chunk(prodp, abp, psSC, psB, ch):
            c0 = ch * D
            # running softmax denominator (broadcast over d-partitions)
            acc = abp.tile([128, D], BF16, tag="acc", name=f"acc{ch}")
            for i, dlt in enumerate(offs):
                prs = [prodp.tile([128, D], BF16, tag=f"pr{j}", name=f"pr{j}_{ch}_{i}")
                       for j in range(DT)]
                for dt_ in range(DT):
                    nc.vector.tensor_tensor(
                        out=prs[dt_][:],
                        in0=ptt[dt_][:, c0 : c0 + D],
                        in1=balt[dt_][:, HALO + c0 + dlt : HALO + c0 + dlt + D],
                        op=AOP.mult,
                    )
                # partition-reduce the 4 d-tiles on the PE (idle here)
                ps = psSC.tile([1, D], F32, tag="pssc")
                for dt_ in range(DT):
                    nc.tensor.matmul(ps[:], ones128[:], prs[dt_][:],
                                     start=(dt_ == 0), stop=(dt_ == DT - 1))
                xrow = p3a.tile([1, D], BF16, tag="xrow")
                nc.scalar.copy(xrow[:], ps[:])
                # broadcast raw scores over the 128 d-partitions (K=1 matmul),
                # then exp via quadratic (|scores| <= ~0.03): e = 1 + x + x^2/2
                psb = psB.tile([128, D], F32, tag="psb")
                nc.tensor.matmul(psb[:], ones1[:], xrow[:], start=True, stop=True)
                # |scores| <= ~0.033 so exp(x) ~ 1+x (softmax weight error
                # <= 6e-4 relative; the rel-term is ~3% of A_out, so this is
                # far below the conv precision floor). The +1 rides the ACT
                # copy's bias port: one ACT op, no DVE work, PSUM freed fast.
                eB = abp.tile([128, D], BF16, tag="eB", name=f"eB_{ch}_{i}")
                nc.scalar.activation(
                    out=eB[:], in_=psb[:], func=ACTF.Copy, bias=1.0, scale=1.0,
                )
                if i == 0:
                    nc.vector.tensor_copy(acc[:], eB[:])
                else:
                    nc.vector.tensor_tensor(out=acc[:], in0=acc[:], in1=eB[:], op=AOP.add)
                # unnormalized ctx accumulation
                for dt_ in range(DT):
                    vsl = vtt[dt_][:, HALO + c0 + dlt : HALO + c0 + dlt + D]
                    if i == 0:
                        nc.vector.tensor_tensor(
                            out=ctxu[dt_][:, c0 : c0 + D], in0=eB[:], in1=vsl,
                            op=AOP.mult,
                        )
                    else:
                        tmp = abp.tile([128, D], BF16, tag=f"tmp{dt_ % 2}")
                        nc.vector.tensor_tensor(out=tmp[:], in0=eB[:], in1=vsl, op=AOP.mult)
                        nc.vector.tensor_tensor(
                            out=ctxu[dt_][:, c0 : c0 + D],
                            in0=ctxu[dt_][:, c0 : c0 + D], in1=tmp[:], op=AOP.add,
                        )
            # normalize by the softmax denominator (in place)
            rb = abp.tile([128, D], F32, tag="rb")
            nc.vector.reciprocal(rb[:], acc[:])
            for dt_ in range(DT):
                nc.vector.tensor_tensor(
                    out=ctxu[dt_][:, c0 : c0 + D],
                    in0=ctxu[dt_][:, c0 : c0 + D], in1=rb[:], op=AOP.mult,
                )

        def emit_3c_t(t):
            psr = psR.tile([128, D], F32, tag="psrel")
            for kd in range(DT):
                nc.tensor.matmul(
                    psr[:], ctxu[kd][:, ts(t, 128)], wotAll[:, kd, :],
                    start=(kd == 0), stop=(kd == DT - 1),
                )
            apb = p3c.tile([128, D], F32, tag="apb")
            nc.sync.dma_start(apb[:], Apb[ts(t, 128), :])
            h = p3c.tile([128, D], F32, tag="h")
            nc.vector.scalar_tensor_tensor(
                out=h[:], in0=psr[:], scalar=0.0, in1=apb[:],
                op0=AOP.bypass, op1=AOP.add,
            )
            st6 = p3c.tile([128, 6], F32, tag="st6")
            nc.vector.bn_stats(out=st6[:], in_=h[:])
            mv = p3c.tile([128, 2], F32, tag="mv")
            nc.vector.bn_aggr(out=mv[:], in_=st6[:])
            sdv = p3c.tile([128, 1], F32, tag="sdv")
            nc.scalar.activation(out=sdv[:], in_=mv[:, 1:2], func=ACTF.Sqrt,
                                 bias=epsS[:], scale=1.0)
            rstd = p3c.tile([128, 1], F32, tag="rstd")
            nc.vector.reciprocal(rstd[:], sdv[:])
            nc.vector.tensor_scalar(
                out=h[:], in0=h[:], scalar1=mv[:, 0:1], scalar2=rstd[:],
                op0=AOP.subtract, op1=AOP.mult,
            )
            nc.vector.tensor_tensor(out=h[:], in0=h[:], in1=gamB[:], op=AOP.mult)
            nc.vector.tensor_tensor(out=h[:], in0=h[:], in1=bet2B[:], op=AOP.add)
            nc.vector.scalar_tensor_tensor(
                out=aout[t][:], in0=apb[:], scalar=flagcS[:], in1=h[:],
                op0=AOP.mult, op1=AOP.add,
            )
            nc.sync.dma_start(A_out[ts(t, 128), :], aout[t][:].bitcast(F32))

        # ---------------- program ----------------
        ct1r = CT1v.rearrange("(kg kk p) j -> kg p kk j", kk=4, p=128)
        # PT projection first (independent of conv1) keeps PE busy early
        with tc.tile_pool(name="at16p", bufs=1) as at16p, \
             tc.tile_pool(name="ps2", bufs=2, space="PSUM") as ps2:
            at16All = at16p.tile([128, DT, HALF], BF16)
            nc.sync.dma_start(at16All[:], AT16.rearrange("(kd p) l -> p kd l", p=128))
            for m in range(DT):
                for c0 in range(0, HALF, D):
                    ps = ps2.tile([128, D], F32, tag="psp")
                    for kd in range(DT):
                        nc.tensor.matmul(
                            ps[:],
                            wqkAll[:, kd, ts(m, 128)],
                            at16All[:, kd, c0 : c0 + D],
                            start=(kd == 0), stop=(kd == DT - 1),
                        )
                    nc.scalar.copy(ptt[m][:, c0 : c0 + D], ps[:])

        with tc.tile_pool(name="prodp", bufs=4) as prodp, \
             tc.tile_pool(name="abp", bufs=3) as abp, \
             tc.tile_pool(name="psSC", bufs=1, space="PSUM") as psSC, \
             tc.tile_pool(name="psB", bufs=1, space="PSUM") as psB, \
             tc.tile_pool(name="psR", bufs=1, space="PSUM") as psR:
            # conv scope closes after the last conv1/VT chunk so conv2 can
            # take all 8 PSUM banks below
            with tc.tile_pool(name="bsbp", bufs=1) as bsbp, \
                 tc.tile_pool(name="ct1", bufs=4) as ct1p, \
                 tc.tile_pool(name="ps1", bufs=1, space="PSUM") as ps1, \
                 tc.tile_pool(name="psV", bufs=1, space="PSUM") as psV:
                bsbAll = bsbp.tile([128, KT, D], BF16)
                nc.sync.dma_start(bsbAll[:], Bin.rearrange("(kt p) d -> p kt d", p=128))
                # interleave: conv1/VT chunks feed attention chunks; Tile
                # overlaps the DVE-heavy attention with the next conv1's PE
                emit_conv1_chunk(ct1p, ps1, bsbAll, ct1r, *C1CH[0])
                emit_conv1_chunk(ct1p, ps1, bsbAll, ct1r, *C1CH[1])
                emit_vt_chunk(psV, *C1CH[0])
                emit_vt_chunk(psV, *C1CH[1])
                emit_attn_chunk(prodp, abp, psSC, psB, 0)
                for t in range(0, 4):
                    emit_3c_t(t)
                emit_conv1_chunk(ct1p, ps1, bsbAll, ct1r, *C1CH[2])
                emit_vt_chunk(psV, *C1CH[2])
                emit_attn_chunk(prodp, abp, psSC, psB, 1)
                for t in range(4, 8):
                    emit_3c_t(t)
                emit_conv1_chunk(ct1p, ps1, bsbAll, ct1r, *C1CH[3])
                emit_vt_chunk(psV, *C1CH[3])
                emit_attn_chunk(prodp, abp, psSC, psB, 2)
                for t in range(8, 12):
                    emit_3c_t(t)
                emit_conv1_chunk(ct1p, ps1, bsbAll, ct1r, *C1CH[4])
                emit_vt_chunk(psV, *C1CH[4])
            emit_attn_chunk(prodp, abp, psSC, psB, 3)
            for t in range(12, 16):
                emit_3c_t(t)

        # ================= conv2: partial B_out =========================
        NCH = L // D
        c2r = C2.rearrange("(kg kk p) l -> kg p kk l", kk=4, p=128)
        with tc.tile_pool(name="c2p", bufs=6) as c2p, \
             tc.tile_pool(name="outp", bufs=4) as outp, \
             tc.tile_pool(name="ps4", bufs=2, space="PSUM") as ps4:
            for nch in range(NCH):
                pss = [ps4.tile([128, D], F32, tag=f"ps4{m}", name=f"ps4{m}") for m in range(DT)]
                for kg in range(NT // 4):
                    c2g = c2p.tile([128, 4, D], F32R, tag="c2")
                    nc.sync.dma_start(c2g[:], c2r[kg, :, :, ts(nch, D)])
                    for kk in range(4):
                        k = kg * 4 + kk
                        for m in range(DT):
                            nc.tensor.matmul(
                                pss[m][:], aout[k][:, ts(m, 128)], c2g[:, kk, :],
                                start=(k == 0), stop=(k == NT - 1),
                            )
                for m in range(DT):
                    osb = outp.tile([128, D], F32, tag="osb")
                    nc.scalar.copy(osb[:], pss[m][:])
                    nc.sync.dma_start(BT_part[ts(m, 128), ts(nch, D)], osb[:])

    _split_excess_waits(nc)
    return nc


_NC_CACHE = {}


def _get_nc():
    if "nc" not in _NC_CACHE:
        _NC_CACHE["nc"] = _build_nc()
    return _NC_CACHE["nc"]


def _gate_flag(A):
    """Replicate reference _gate on host (fp64; decision margin is ~0.7)."""
    A = np.asarray(A, np.float64)
    pooled = A.mean(axis=1)
    sims = pooled @ pooled.T
    sims = sims - np.eye(sims.shape[0]) * 1e9
    srt = np.sort(sims, axis=-1)
    margin = srt[:, -1] - srt[:, -2]
    m = sims.max(axis=-1, keepdims=True)
    logp = sims - m - np.log(np.exp(sims - m).sum(axis=-1, keepdims=True))
    probs = np.exp(logp)
    entropy = -(probs * np.log(probs + 1e-9)).sum(axis=-1)
    confidence = margin - 0.5 * entropy
    fire = confidence < THRESHOLD
    return bool(fire.any())


def _circulant(w):
    """C1[l, m] = w[(l - m) % L] as float32."""
    v = w[::-1].astype(np.float32)
    big = np.concatenate([v, v])
    S = np.lib.stride_tricks.sliding_window_view(big, L)  # S[s] = big[s:s+L]
    return np.ascontiguousarray(S[L - 1 - np.arange(L)])


def kernel(A, B, phase_weights, Wq, Wk, Wv, Wo, bo, ln_scale, ln_bias):
    A = np.asarray(A, np.float32)
    B = np.asarray(B, np.float32)
    phase_weights = np.asarray(phase_weights, np.float32)
    Wq, Wk, Wv, Wo = (np.asarray(x, np.float32) for x in (Wq, Wk, Wv, Wo))
    bo = np.asarray(bo, np.float32)
    ln_scale = np.asarray(ln_scale, np.float32)
    ln_bias = np.asarray(ln_bias, np.float32)

    nc = _get_nc()

    pw = phase_weights.astype(np.float64)
    wv = np.exp(pw - pw.max())
    wv = (wv / wv.sum()).astype(np.float32)
    C1 = _circulant(wv)  # (L, L) f32

    flag = 1.0 if _gate_flag(A) else 0.0
    flagc = np.float32(1.0 - flag)
    gam = (flag * ln_scale).astype(np.float32)
    bet2 = (flag * ln_bias - flagc * bo).astype(np.float32)

    Wqk = ((Wq.T @ Wk) / np.sqrt(np.float32(D))).astype(ml_dtypes.bfloat16)
    WvT = Wv.T.astype(ml_dtypes.bfloat16)
    WoT = Wo.T.astype(ml_dtypes.bfloat16)

    in_maps = []
    for b in range(Bsz):
        for h in range(2):
            own0 = h * HALF
            rows = (own0 - HALO + np.arange(WID)) % L
            CT1v_np = np.ascontiguousarray(C1[rows].T).astype(ml_dtypes.bfloat16)
            in_maps.append({
                "Bin": B[b].astype(ml_dtypes.bfloat16),
                "CT1v": CT1v_np,
                "C2": np.ascontiguousarray(C1[own0 : own0 + HALF]),
                "AT16": np.ascontiguousarray(A[b, own0 : own0 + HALF].T).astype(
                    ml_dtypes.bfloat16
                ),
                "Apb": A[b, own0 : own0 + HALF] + bo,
                "Wqk": Wqk,
                "WvT": WvT,
                "WoT": WoT,
                "gam": gam,
                "bet2": bet2,
                "flagc": np.array([flagc], np.float32),
            })

    res = run_bass_kernel_spmd(nc, in_maps, core_ids=list(range(8)))

    A_out = np.empty((Bsz, L, D), np.float32)
    B_out = np.empty((Bsz, L, D), np.float32)
    for b in range(Bsz):
        r0 = res.results[2 * b]
        r1 = res.results[2 * b + 1]
        A_out[b, :HALF] = r0["A_out"]
        A_out[b, HALF:] = r1["A_out"]
        B_out[b] = (r0["BT_part"] + r1["BT_part"]).T
    return A_out, B_out



# revision 8
# speedup vs baseline: 1.7601x; 1.7601x over previous
"""Trainium2 Bass kernel for nn_CrossResonanceLayer (sparse_attention).

Math (reference):
  w  = softmax(phase_weights)                          (L,)
  B_aligned = circconv(B, w)          = C1 @ B[b]      C1[l,m] = w[(l-m)%L]
  fire = gate(A)  -> scalar flag (host, tiny BxB math on pooled vectors)
  windowed local attention (radius 4) on (A, B_aligned), layernorm(A + rel)
  A_out = flag ? normed : A
  B_out = circconv(A_out, roll(w[::-1],1)) = C1^T @ A_out[b]

Sharding: 8 cores = (batch b in 0..3) x (sequence half h in 0..1), as in the
baseline.  Host sums the two conv2 partials per batch; no collectives.

Key implementation points (v2):
 * Both circulant matmuls (conv1/conv2) run in fp8e4 with DoubleRow perf
   mode (2 contraction rows per cycle).  The circulant is split
   C1 = (1/L)*ones + Delta: the rank-1 mean part becomes a per-partition
   scalar correction, and only the small residual delta (scaled to the fp8
   dynamic range) is quantized, keeping the weight quantization error
   ~50x below quantizing w directly.
 * Circulant operand tiles are never streamed from HBM: every moving
   operand of conv1/conv2 is a slice of a single SBUF-resident
   [128, 2, M] "generator" image of delta (host-precomputed), cutting
   ~50MB of per-core DMA traffic to ~3.5MB.
 * Attention is computed with PE matmuls instead of DVE broadcasts:
   a banded [128 l, 136 j] score block per 128-row tile (4 matmuls),
   row-major softmax (per-partition scalars, exp(x)~=1+x), PE transpose of
   the tiny attention block, then ctx^T = V_rows^T @ attn^T directly in
   feature-major layout for the Wo matmul.  This removes ~150us of DVE
   work vs the broadcast formulation.
 * Elementwise work is spread across DVE / ACT / GPSIMD.
"""
import sys

sys.path.insert(0, "/opt/trn_rl_repo")

from contextlib import ExitStack

import numpy as np
import ml_dtypes

import concourse.bass as bass
import concourse.tile as tile
from concourse import mybir
from concourse.bass_utils import run_bass_kernel_spmd
from concourse.masks import make_identity

F32 = mybir.dt.float32
BF16 = mybir.dt.bfloat16
FP8 = mybir.dt.float8e4
AOP = mybir.AluOpType
ACTF = mybir.ActivationFunctionType
DR = mybir.MatmulPerfMode.DoubleRow

Bsz, L, D = 4, 4096, 512
HALF = L // 2              # 2048 rows per core
HALO = 8                   # windowed attention needs only +-4
WID = HALF + 2 * HALO      # 2064 halo-extended rows
NT = HALF // 128           # 16 own l-tiles
KT = L // 128              # 32 k-tiles along L
DT = D // 128              # 4 d-tiles
RADIUS = 4
W9 = 2 * RADIUS + 1        # 9
BW = 128 + 2 * RADIUS      # 136 banded score width
LN_EPS = 1e-5
THRESHOLD = 0.15

Q1MIN = -(L - 256)         # -3840 (conv1 contracts over L: 16 k-pairs)
M1 = WID - Q1MIN           # 5904
Q2MIN = -(HALF - 256)      # -1792 (conv2 contracts over HALF: 8 k-pairs)
M2 = L - Q2MIN             # 5888


def _split_excess_waits(nc, max_waits=1):
    """This walrus build accepts at most one sem-wait command per instruction.
    Move excess waits onto same-engine NOPs placed right before the owner."""
    ctr = 0
    for fn in nc.m.functions:
        for bb in fn.blocks:
            out = []
            changed = False
            for inst in bb.instructions:
                si = inst.sync_info
                if si is not None and len(si.on_wait) > max_waits:
                    waits = list(si.on_wait)
                    keep = waits[-max_waits:]
                    extra = waits[:-max_waits]
                    for i in range(0, len(extra), max_waits):
                        nop = mybir.InstNoOp(name=f"waitsplit-{ctr}")
                        ctr += 1
                        nop.engine = inst.engine
                        nop.sync_info = mybir.SyncInfo(
                            on_wait=extra[i : i + max_waits], on_update=[]
                        )
                        out.append(nop)
                    si.on_wait = keep
                    changed = True
                out.append(inst)
            if changed:
                bb.instructions = out
    return ctr


def _build_nc():
    nc = bass.Bass("TRN2", target_bir_lowering=False, debug=False, num_devices=8)

    # ---- inputs (per core) ----
    Bin8 = nc.dram_tensor("Bin8", [L, D], FP8, kind="ExternalInput").ap()
    WR1 = nc.dram_tensor("WR1", [128, 2, M1], FP8, kind="ExternalInput").ap()
    WR2 = nc.dram_tensor("WR2", [128, 2, M2], FP8, kind="ExternalInput").ap()
    AT16 = nc.dram_tensor("AT16", [D, HALF], BF16, kind="ExternalInput").ap()
    Apb = nc.dram_tensor("Apb", [HALF, D], F32, kind="ExternalInput").ap()  # A + bo
    Wqk = nc.dram_tensor("Wqk", [D, D], BF16, kind="ExternalInput").ap()   # WqT@Wk/(sqrt(d)*4096)
    WvT = nc.dram_tensor("WvT", [D, D], BF16, kind="ExternalInput").ap()   # Wv.T/4096
    WoT = nc.dram_tensor("WoT", [D, D], BF16, kind="ExternalInput").ap()
    Scol = nc.dram_tensor("Scol", [128, DT], F32, kind="ExternalInput").ap()  # colsum(B) per d
    Mask = nc.dram_tensor("Mask", [128, BW], BF16, kind="ExternalInput").ap()
    gam = nc.dram_tensor("gam", [D], BF16, kind="ExternalInput").ap()   # flag*ln_scale
    bet2 = nc.dram_tensor("bet2", [D], BF16, kind="ExternalInput").ap() # flag*ln_bias-(1-flag)*bo
    flagc = nc.dram_tensor("flagc", [1], F32, kind="ExternalInput").ap()  # 1-flag
    scal1 = nc.dram_tensor("scal1", [1], F32, kind="ExternalInput").ap()  # 4096/SD1
    scal2 = nc.dram_tensor("scal2", [1], F32, kind="ExternalInput").ap()  # 1/SD2

    # ---- outputs ----
    A_out = nc.dram_tensor("A_out", [HALF, D], F32, kind="ExternalOutput").ap()
    BT_part = nc.dram_tensor("BT_part", [D, L], F32, kind="ExternalOutput").ap()

    def bcast(row_ap, parts=128):
        return bass.AP(
            tensor=row_ap.tensor,
            offset=row_ap.offset,
            ap=[[0, parts]] + list(row_ap.ap),
        )

    ts = bass.ts
    C1CH = [(c, min(D, WID - c)) for c in range(0, WID, D)]  # conv1 chunks

    with tile.TileContext(nc) as tc, ExitStack() as ctx:
        consts = ctx.enter_context(tc.tile_pool(name="consts", bufs=1))
        gamB = consts.tile([128, D], BF16)
        nc.sync.dma_start(gamB[:], bcast(gam))
        bet2B = consts.tile([128, D], BF16)
        nc.sync.dma_start(bet2B[:], bcast(bet2))
        flagcS = consts.tile([128, 1], F32)
        nc.sync.dma_start(flagcS[:], bcast(flagc))
        scal1S = consts.tile([128, 1], F32)
        nc.sync.dma_start(scal1S[:], bcast(scal1))
        scal2S = consts.tile([128, 1], F32)
        nc.sync.dma_start(scal2S[:], bcast(scal2))
        epsS = consts.tile([128, 1], F32)
        nc.vector.memset(epsS[:], LN_EPS)
        maskT = consts.tile([128, BW], BF16)
        nc.sync.dma_start(maskT[:], Mask)
        ScolT = consts.tile([128, DT], F32)
        nc.sync.dma_start(ScolT[:], Scol)
        ident = consts.tile([128, 128], BF16)
        make_identity(nc, ident[:])
        ones8 = consts.tile([128, 2, 1], FP8)
        nc.vector.memset(ones8[:], 1.0)

        wpool = ctx.enter_context(tc.tile_pool(name="wpool", bufs=1))
        wqkAll = wpool.tile([128, DT, D], BF16)
        nc.sync.dma_start(wqkAll[:], Wqk.rearrange("(kd p) d -> p kd d", p=128))
        wvtAll = wpool.tile([128, DT, D], BF16)
        nc.sync.dma_start(wvtAll[:], WvT.rearrange("(kd p) d -> p kd d", p=128))
        wotAll = wpool.tile([128, DT, D], BF16)
        nc.sync.dma_start(wotAll[:], WoT.rearrange("(kd p) d -> p kd d", p=128))
        wr2 = wpool.tile([128, 2, M2], FP8)
        nc.sync.dma_start(wr2[:], WR2)

        # persistent activations
        persist = ctx.enter_context(tc.tile_pool(name="persist", bufs=1))
        ptt = persist.tile([128, DT, HALF], BF16)   # (A Wqk)^T feature-major
        balt = persist.tile([128, DT, WID], BF16)   # 4096*B_al^T feature-major
        vrows = persist.tile([128, NT + 1, D], BF16)  # V rows, shifted by -4
        aout8 = persist.tile([128, NT, D], FP8)     # A_out quantized for conv2

        # ---------------- PT projection + conv1 input loads ----------------
        with tc.tile_pool(name="c1in", bufs=1) as c1in:
            bsbAll = c1in.tile([128, KT, D], FP8)
            nc.sync.dma_start(bsbAll[:], Bin8.rearrange("(kt p) d -> p kt d", p=128))
            wr1 = c1in.tile([128, 2, M1], FP8)
            nc.sync.dma_start(wr1[:], WR1)
            with tc.tile_pool(name="at16p", bufs=1) as at16p, \
                 tc.tile_pool(name="ps2", bufs=2, space="PSUM") as ps2:
                at16All = at16p.tile([128, DT, HALF], BF16)
                nc.sync.dma_start(at16All[:],
                                  AT16.rearrange("(kd p) l -> p kd l", p=128))
                for m in range(DT):
                    for c0 in range(0, HALF, D):
                        ps = ps2.tile([128, D], F32, tag="psp")
                        for kd in range(DT):
                            nc.tensor.matmul(
                                ps[:],
                                wqkAll[:, kd, ts(m, 128)],
                                at16All[:, kd, c0 : c0 + D],
                                start=(kd == 0), stop=(kd == DT - 1),
                            )
                        nc.scalar.copy(ptt[:, m, c0 : c0 + D], ps[:])

            # ---------------- conv1 + attention interleave ----------------
            with tc.tile_pool(name="ps1", bufs=1, space="PSUM") as ps1, \
                 tc.tile_pool(name="psV", bufs=1, space="PSUM") as psV, \
                 tc.tile_pool(name="psS", bufs=1, space="PSUM") as psS, \
                 tc.tile_pool(name="psT", bufs=1, space="PSUM") as psT, \
                 tc.tile_pool(name="psC", bufs=2, space="PSUM") as psC, \
                 tc.tile_pool(name="psR", bufs=1, space="PSUM") as psR, \
                 tc.tile_pool(name="smp", bufs=2) as smp, \
                 tc.tile_pool(name="atp", bufs=2) as atp, \
                 tc.tile_pool(name="ctp", bufs=2) as ctp, \
                 tc.tile_pool(name="p3c", bufs=2) as p3c:

                def emit_conv1_chunk(c0, cw):
                    for m in range(DT):
                        ps = ps1.tile([128, D], F32, tag="ps1")
                        for k in range(KT // 2):
                            q0 = c0 - 256 * k - Q1MIN
                            nc.tensor.matmul(
                                ps[:, 0:cw],
                                bsbAll[:, 2 * k : 2 * k + 2, ts(m, 128)],
                                wr1[:, :, q0 : q0 + cw],
                                start=(k == 0), stop=(k == KT // 2 - 1),
                                perf_mode=DR,
                            )
                        # balt = ps * (4096/SD1) + colsum(B)[d]  (mean part)
                        nc.vector.tensor_scalar(
                            out=balt[:, m, c0 : c0 + cw], in0=ps[:, 0:cw],
                            scalar1=scal1S[:], scalar2=ScolT[:, m : m + 1],
                            op0=AOP.mult, op1=AOP.add,
                        )

                def emit_vrow(i):
                    nr = 128 if i < NT else 8  # tile NT holds only 8 halo rows
                    ps = psV.tile([128, D], F32, tag="psv")
                    for kd in range(DT):
                        nc.tensor.matmul(
                            ps[0:nr, :],
                            balt[:, kd, i * 128 + 4 : i * 128 + 4 + nr],
                            wvtAll[:, kd, :],
                            start=(kd == 0), stop=(kd == DT - 1),
                        )
                    nc.scalar.copy(vrows[0:nr, i, :], ps[0:nr, :])

                def emit_attn(t):
                    # banded scores [128 l, 136 j] on the PE
                    ps_s = psS.tile([128, BW], F32, tag="pss")
                    for kd in range(DT):
                        nc.tensor.matmul(
                            ps_s[:],
                            ptt[:, kd, ts(t, 128)],
                            balt[:, kd, t * 128 + 4 : t * 128 + 4 + BW],
                            start=(kd == 0), stop=(kd == DT - 1),
                        )
                    # softmax with exp(x) ~= 1+x (|s| <= ~0.04)
                    sm = smp.tile([128, BW], BF16, tag="sm")
                    nc.vector.tensor_tensor(out=sm[:], in0=ps_s[:], in1=maskT[:],
                                            op=AOP.mult)
                    rs = smp.tile([128, 1], F32, tag="rs")
                    nc.vector.tensor_reduce(out=rs[:], in_=sm[:],
                                            axis=mybir.AxisListType.X, op=AOP.add)
                    rs9 = smp.tile([128, 1], F32, tag="rs9")
                    nc.vector.tensor_scalar(out=rs9[:], in0=rs[:], scalar1=float(W9),
                                            scalar2=None, op0=AOP.add)
                    racc = smp.tile([128, 1], F32, tag="racc")
                    nc.vector.reciprocal(racc[:], rs9[:])
                    sm1 = smp.tile([128, BW], BF16, tag="sm1")
                    nc.vector.tensor_tensor(out=sm1[:], in0=sm[:], in1=maskT[:],
                                            op=AOP.add)
                    attnw = smp.tile([128, BW], BF16, tag="attnw")
                    nc.vector.tensor_scalar(out=attnw[:], in0=sm1[:], scalar1=racc[:],
                                            scalar2=None, op0=AOP.mult)
                    # transpose the attention block
                    pT1 = psT.tile([128, 128], BF16, tag="pt1")
                    nc.tensor.transpose(pT1[:], attnw[:, 0:128], ident[:])
                    pT2 = psT.tile([8, 128], BF16, tag="pt2")
                    nc.tensor.transpose(pT2[:], attnw[:, 128:BW], ident[:])
                    aT1 = atp.tile([128, 128], BF16, tag="at1")
                    nc.vector.tensor_copy(aT1[:], pT1[:])
                    aT2 = atp.tile([8, 128], BF16, tag="at2")
                    nc.vector.tensor_copy(aT2[:], pT2[:])
                    # ctx^T (feature-major) = V_rows^T @ attn^T
                    ctile = ctp.tile([128, DT, 128], BF16, tag="ct")
                    for dt_ in range(DT):
                        pc = psC.tile([128, 128], F32, tag="pc")
                        nc.tensor.matmul(pc[:], vrows[:, t, ts(dt_, 128)], aT1[:],
                                         start=True, stop=False)
                        nc.tensor.matmul(pc[:], vrows[0:8, t + 1, ts(dt_, 128)],
                                         aT2[:], start=False, stop=True)
                        nc.scalar.copy(ctile[:, dt_, :], pc[:])
                    # rel + layernorm
                    psr = psR.tile([128, D], F32, tag="psrel")
                    for kd in range(DT):
                        nc.tensor.matmul(
                            psr[:], ctile[:, kd, :], wotAll[:, kd, :],
                            start=(kd == 0), stop=(kd == DT - 1),
                        )
                    apbt = p3c.tile([128, D], F32, tag="apb")
                    nc.sync.dma_start(apbt[:], Apb[ts(t, 128), :])
                    h = p3c.tile([128, D], BF16, tag="h")
                    nc.vector.scalar_tensor_tensor(
                        out=h[:], in0=psr[:], scalar=0.0, in1=apbt[:],
                        op0=AOP.bypass, op1=AOP.add,
                    )
                    st6 = p3c.tile([128, 6], F32, tag="st6")
                    nc.vector.bn_stats(out=st6[:], in_=h[:])
                    mv = p3c.tile([128, 2], F32, tag="mv")
                    nc.vector.bn_aggr(out=mv[:], in_=st6[:])
                    sdv = p3c.tile([128, 1], F32, tag="sdv")
                    nc.scalar.activation(out=sdv[:], in_=mv[:, 1:2], func=ACTF.Sqrt,
                                         bias=epsS[:], scale=1.0)
                    rstd = p3c.tile([128, 1], F32, tag="rstd")
                    nc.vector.reciprocal(rstd[:], sdv[:])
                    hn = p3c.tile([128, D], BF16, tag="hn")
                    nc.vector.tensor_scalar(
                        out=hn[:], in0=h[:], scalar1=mv[:, 0:1], scalar2=rstd[:],
                        op0=AOP.subtract, op1=AOP.mult,
                    )
                    hg = p3c.tile([128, D], BF16, tag="hg")
                    nc.gpsimd.tensor_tensor(out=hg[:], in0=hn[:], in1=gamB[:],
                                            op=AOP.mult)
                    hb = p3c.tile([128, D], BF16, tag="hb")
                    nc.gpsimd.tensor_tensor(out=hb[:], in0=hg[:], in1=bet2B[:],
                                            op=AOP.add)
                    aoutt = p3c.tile([128, D], F32, tag="aout")
                    nc.vector.scalar_tensor_tensor(
                        out=aoutt[:], in0=apbt[:], scalar=flagcS[:], in1=hb[:],
                        op0=AOP.mult, op1=AOP.add,
                    )
                    nc.sync.dma_start(A_out[ts(t, 128), :], aoutt[:])
                    nc.scalar.copy(aout8[:, t, :], aoutt[:])

                emit_conv1_chunk(*C1CH[0])
                emit_conv1_chunk(*C1CH[1])
                for i in range(0, 7):
                    emit_vrow(i)
                for t in range(0, 6):
                    emit_attn(t)
                emit_conv1_chunk(*C1CH[2])
                for i in range(7, 11):
                    emit_vrow(i)
                for t in range(6, 10):
                    emit_attn(t)
                emit_conv1_chunk(*C1CH[3])
                for i in range(11, 15):
                    emit_vrow(i)
                for t in range(10, 14):
                    emit_attn(t)
                emit_conv1_chunk(*C1CH[4])
                for i in range(15, 17):
                    emit_vrow(i)
                for t in range(14, 16):
                    emit_attn(t)

        # ================= conv2: partial B_out =========================
        NCH = L // D
        with tc.tile_pool(name="outp", bufs=4) as outp, \
             tc.tile_pool(name="ps4", bufs=4, space="PSUM") as ps4, \
             tc.tile_pool(name="psTm", bufs=1, space="PSUM") as psTm:
            # per-d column-sum of own A_out rows (mean part of C2)
            Tm = outp.tile([128, DT], F32, tag="tm")
            for m in range(DT):
                pt_ = psTm.tile([128, 1], F32, tag="ptm")
                for k in range(NT // 2):
                    nc.tensor.matmul(
                        pt_[:], aout8[:, 2 * k : 2 * k + 2, ts(m, 128)], ones8[:],
                        start=(k == 0), stop=(k == NT // 2 - 1), perf_mode=DR,
                    )
                # Tm = colsum/4096 (the ubar factor)
                nc.scalar.activation(out=Tm[:, m : m + 1], in_=pt_[:],
                                     func=ACTF.Copy, scale=1.0 / L)
            for nch in range(NCH):
                for m in range(DT):
                    ps = ps4.tile([128, D], F32, tag="ps4")
                    for k in range(NT // 2):
                        q0 = nch * D - 256 * k - Q2MIN
                        nc.tensor.matmul(
                            ps[:], aout8[:, 2 * k : 2 * k + 2, ts(m, 128)],
                            wr2[:, :, q0 : q0 + D],
                            start=(k == 0), stop=(k == NT // 2 - 1), perf_mode=DR,
                        )
                    osb = outp.tile([128, D], F32, tag="osb")
                    nc.vector.tensor_scalar(
                        out=osb[:], in0=ps[:], scalar1=scal2S[:],
                        scalar2=Tm[:, m : m + 1], op0=AOP.mult, op1=AOP.add,
                    )
                    nc.sync.dma_start(BT_part[ts(m, 128), ts(nch, D)], osb[:])

    _split_excess_waits(nc)
    return nc


_NC_CACHE = {}


def _get_nc():
    if "nc" not in _NC_CACHE:
        _NC_CACHE["nc"] = _build_nc()
    return _NC_CACHE["nc"]


def _gate_flag(A):
    """Replicate reference _gate on host (fp64; decision margin is ~0.7)."""
    A = np.asarray(A, np.float64)
    pooled = A.mean(axis=1)
    sims = pooled @ pooled.T
    sims = sims - np.eye(sims.shape[0]) * 1e9
    srt = np.sort(sims, axis=-1)
    margin = srt[:, -1] - srt[:, -2]
    m = sims.max(axis=-1, keepdims=True)
    logp = sims - m - np.log(np.exp(sims - m).sum(axis=-1, keepdims=True))
    probs = np.exp(logp)
    entropy = -(probs * np.log(probs + 1e-9)).sum(axis=-1)
    confidence = margin - 0.5 * entropy
    fire = confidence < THRESHOLD
    return bool(fire.any())


def kernel(A, B, phase_weights, Wq, Wk, Wv, Wo, bo, ln_scale, ln_bias):
    A = np.asarray(A, np.float32)
    B = np.asarray(B, np.float32)
    phase_weights = np.asarray(phase_weights, np.float32)
    Wq, Wk, Wv, Wo = (np.asarray(x, np.float32) for x in (Wq, Wk, Wv, Wo))
    bo = np.asarray(bo, np.float32)
    ln_scale = np.asarray(ln_scale, np.float32)
    ln_bias = np.asarray(ln_bias, np.float32)

    nc = _get_nc()

    pw = phase_weights.astype(np.float64)
    wv = np.exp(pw - pw.max())
    wv = wv / wv.sum()                      # softmax weights, fp64
    ubar = 1.0 / L
    delta = wv - ubar
    dmax = max(np.abs(delta).max(), 1e-30)
    SD1 = 192.0 / dmax                      # scale residual into fp8 range
    SD2 = SD1
    d8 = (delta * SD1).astype(ml_dtypes.float8_e4m3)

    flag = 1.0 if _gate_flag(A) else 0.0
    flagc = np.float32(1.0 - flag)
    gam16 = (flag * ln_scale).astype(ml_dtypes.bfloat16)
    bet2_16 = (flag * ln_bias - flagc * bo).astype(ml_dtypes.bfloat16)

    Wqk = ((Wq.T @ Wk) / (np.sqrt(np.float32(D)) * L)).astype(ml_dtypes.bfloat16)
    WvT = (Wv.T / L).astype(ml_dtypes.bfloat16)
    WoT = Wo.T.astype(ml_dtypes.bfloat16)

    # band mask [128, 136]: row p attends to j in [p, p+8]
    jj = np.arange(BW)[None, :]
    pp = np.arange(128)[:, None]
    mask = ((jj >= pp) & (jj <= pp + 2 * RADIUS)).astype(ml_dtypes.bfloat16)

    # fp8 generator images of the delta-circulant, one per half
    p_ = np.arange(128)[:, None, None]
    i_ = np.arange(2)[None, :, None]
    wr1s, wr2s = [], []
    for h in range(2):
        own0 = h * HALF
        m1 = np.arange(M1)[None, None, :]
        idx1 = (own0 - HALO + (m1 + Q1MIN) - 128 * i_ - p_) % L
        wr1s.append(d8[idx1])
        m2 = np.arange(M2)[None, None, :]
        idx2 = (own0 + 128 * i_ + p_ - (m2 + Q2MIN)) % L
        wr2s.append(d8[idx2])

    in_maps = []
    for b in range(Bsz):
        Scol_b = B[b].sum(axis=0).astype(np.float32).reshape(DT, 128).T
        Scol_b = np.ascontiguousarray(Scol_b)
        B8 = B[b].astype(ml_dtypes.float8_e4m3)
        for h in range(2):
            own0 = h * HALF
            in_maps.append({
                "Bin8": B8,
                "WR1": wr1s[h],
                "WR2": wr2s[h],
                "AT16": np.ascontiguousarray(A[b, own0 : own0 + HALF].T).astype(
                    ml_dtypes.bfloat16
                ),
                "Apb": A[b, own0 : own0 + HALF] + bo,
                "Wqk": Wqk,
                "WvT": WvT,
                "WoT": WoT,
                "Scol": Scol_b,
                "Mask": mask,
                "gam": gam16,
                "bet2": bet2_16,
                "flagc": np.array([flagc], np.float32),
                "scal1": np.array([L / SD1], np.float32),
                "scal2": np.array([1.0 / SD2], np.float32),
            })

    res = run_bass_kernel_spmd(nc, in_maps, core_ids=list(range(8)))

    A_out = np.empty((Bsz, L, D), np.float32)
    B_out = np.empty((Bsz, L, D), np.float32)
    for b in range(Bsz):
        r0 = res.results[2 * b]
        r1 = res.results[2 * b + 1]
        A_out[b, :HALF] = r0["A_out"]
        A_out[b, HALF:] = r1["A_out"]
        B_out[b] = (r0["BT_part"] + r1["BT_part"]).T
    return A_out, B_out


# revision 11
# speedup vs baseline: 1.8333x; 1.0416x over previous
"""Trainium2 Bass kernel for nn_CrossResonanceLayer (sparse_attention).

Math (reference):
  w  = softmax(phase_weights)                          (L,)
  B_aligned = circconv(B, w)          = C1 @ B[b]      C1[l,m] = w[(l-m)%L]
  fire = gate(A)  -> scalar flag (host, tiny BxB math on pooled vectors)
  windowed local attention (radius 4) on (A, B_aligned), layernorm(A + rel)
  A_out = flag ? normed : A
  B_out = circconv(A_out, roll(w[::-1],1)) = C1^T @ A_out[b]

Sharding: 8 cores = (batch b in 0..3) x (sequence half h in 0..1).  Host sums
the two conv2 partials per batch; no collectives.

Implementation highlights:
 * The circulant is split C1 = (1/L)*ones + Delta.  The rank-1 mean part is
   applied as an exact per-partition scalar correction (host colsums), so
   only the small residual delta runs on the PE -- in fp8e4 DoubleRow mode
   (2 contraction rows/cycle).  Quantizing delta (2% of the weight mass,
   scaled to fp8 range) keeps conv weight error ~50x below quantizing w.
 * conv1/conv2 moving operands are slices of a single SBUF-resident
   [128, 2, M] generator image of delta; no circulant HBM traffic at all.
 * The q/o projections also run as fp8 DoubleRow matmuls with build-time
   power-of-two scale folding (K1/K2), descaled for free in later ops.
 * Attention is PE-centric: banded [128 l, 136 j] score block per tile,
   row-major softmax with exp(x)~=1+x and per-partition scalars, PE
   transpose of the attention block, ctx^T = V_rows^T @ attn^T directly in
   feature-major layout for the Wo matmul.
 * Elementwise work is spread across DVE / ACT / GPSIMD; large input DMAs
   are issued from otherwise-idle engine queues in need-order.
"""
import sys

sys.path.insert(0, "/opt/trn_rl_repo")

from contextlib import ExitStack

import numpy as np
import ml_dtypes

import concourse.bass as bass
import concourse.tile as tile
from concourse import mybir
from concourse.bass_utils import run_bass_kernel_spmd
from concourse.masks import make_identity

F32 = mybir.dt.float32
BF16 = mybir.dt.bfloat16
FP8 = mybir.dt.float8e4
AOP = mybir.AluOpType
ACTF = mybir.ActivationFunctionType
DR = mybir.MatmulPerfMode.DoubleRow

Bsz, L, D = 4, 4096, 512
HALF = L // 2              # 2048 rows per core
HALO = 8                   # windowed attention needs only +-4
WID = HALF + 2 * HALO      # 2064 halo-extended rows
NT = HALF // 128           # 16 own l-tiles
KT = L // 128              # 32 k-tiles along L
DT = D // 128              # 4 d-tiles
RADIUS = 4
W9 = 2 * RADIUS + 1        # 9
BW = 128 + 2 * RADIUS      # 136 banded score width
LN_EPS = 1e-5
THRESHOLD = 0.15

Q1MIN = -(L - 256)         # -3840 (conv1 contracts over L: 16 k-pairs)
M1 = WID - Q1MIN           # 5904
Q2MIN = -(HALF - 256)      # -1792 (conv2 contracts over HALF: 8 k-pairs)
M2 = L - Q2MIN             # 5888

K1 = 256.0                 # build-time fp8 scale for Wqk
K2 = 64.0                  # build-time fp8 scale for WoT
CT_S = 64.0                # build-time fp8 scale for ctx tiles
# conv1 chunk widths (<=512 for one PSUM bank, no tiny tail chunk)
C1CH = [(0, 416), (416, 416), (832, 416), (1248, 416), (1664, 400)]


def _split_excess_waits(nc, max_waits=1):
    """This walrus build accepts at most one sem-wait command per instruction.
    Move excess waits onto same-engine NOPs placed right before the owner."""
    ctr = 0
    for fn in nc.m.functions:
        for bb in fn.blocks:
            out = []
            changed = False
            for inst in bb.instructions:
                si = inst.sync_info
                if si is not None and len(si.on_wait) > max_waits:
                    waits = list(si.on_wait)
                    keep = waits[-max_waits:]
                    extra = waits[:-max_waits]
                    for i in range(0, len(extra), max_waits):
                        nop = mybir.InstNoOp(name=f"waitsplit-{ctr}")
                        ctr += 1
                        nop.engine = inst.engine
                        nop.sync_info = mybir.SyncInfo(
                            on_wait=extra[i : i + max_waits], on_update=[]
                        )
                        out.append(nop)
                    si.on_wait = keep
                    changed = True
                out.append(inst)
            if changed:
                bb.instructions = out
    return ctr


def _build_nc():
    nc = bass.Bass("TRN2", target_bir_lowering=False, debug=False, num_devices=8)

    # ---- inputs (per core) ----
    Bin8 = nc.dram_tensor("Bin8", [L, D], FP8, kind="ExternalInput").ap()
    WR1 = nc.dram_tensor("WR1", [128, 2, M1], FP8, kind="ExternalInput").ap()
    WR2 = nc.dram_tensor("WR2", [128, 2, M2], FP8, kind="ExternalInput").ap()
    AT8 = nc.dram_tensor("AT8", [D, HALF], FP8, kind="ExternalInput").ap()
    Apb = nc.dram_tensor("Apb", [HALF, D], F32, kind="ExternalInput").ap()  # A + bo
    Wqk8 = nc.dram_tensor("Wqk8", [D, D], FP8, kind="ExternalInput").ap()  # K1*WqT@Wk/sqrt(d)
    WvT = nc.dram_tensor("WvT", [D, D], BF16, kind="ExternalInput").ap()   # Wv.T/L
    WoT8 = nc.dram_tensor("WoT8", [D, D], FP8, kind="ExternalInput").ap()  # K2*Wo.T
    Scol = nc.dram_tensor("Scol", [128, DT], F32, kind="ExternalInput").ap()  # colsum(B)
    MaskC = nc.dram_tensor("MaskC", [128, BW], BF16, kind="ExternalInput").ap()
    MaskB = nc.dram_tensor("MaskB", [128, BW], BF16, kind="ExternalInput").ap()
    gam = nc.dram_tensor("gam", [D], BF16, kind="ExternalInput").ap()   # flag*ln_scale
    bet2 = nc.dram_tensor("bet2", [D], BF16, kind="ExternalInput").ap() # flag*ln_bias-(1-flag)*bo
    flagc = nc.dram_tensor("flagc", [1], F32, kind="ExternalInput").ap()  # 1-flag
    scal1 = nc.dram_tensor("scal1", [1], F32, kind="ExternalInput").ap()  # L/SD1

    # ---- outputs ----
    A_out = nc.dram_tensor("A_out", [HALF, D], F32, kind="ExternalOutput").ap()
    BT_part = nc.dram_tensor("BT_part", [D, L], F32, kind="ExternalOutput").ap()

    def bcast(row_ap, parts=128):
        return bass.AP(
            tensor=row_ap.tensor,
            offset=row_ap.offset,
            ap=[[0, parts]] + list(row_ap.ap),
        )

    ts = bass.ts

    with tile.TileContext(nc) as tc, ExitStack() as ctx:
        # persistent activations
        persist = ctx.enter_context(tc.tile_pool(name="persist", bufs=1))
        ptt = persist.tile([128, DT, HALF], BF16)   # K1*4096*(A Wqk)^T feature-major
        balt = persist.tile([128, DT, WID], BF16)   # 4096*B_al^T feature-major
        vrows = persist.tile([128, NT + 1, D], BF16)  # V rows, shifted by -4
        aout8 = persist.tile([128, NT, D], FP8)     # A_out quantized for conv2

        wpool = ctx.enter_context(tc.tile_pool(name="wpool", bufs=1))
        # issue big loads from idle engine queues, in need-order
        wqk8All = wpool.tile([128, DT, D], FP8)
        nc.sync.dma_start(wqk8All[:], Wqk8.rearrange("(kd p) d -> p kd d", p=128))
        at8All = wpool.tile([128, DT, HALF], FP8)
        nc.sync.dma_start(at8All[:], AT8.rearrange("(kd p) l -> p kd l", p=128))
        bsbAll = wpool.tile([128, KT, D], FP8)
        nc.gpsimd.dma_start(bsbAll[:], Bin8.rearrange("(kt p) d -> p kt d", p=128))
        wr1 = wpool.tile([128, 2, M1], FP8)
        nc.scalar.dma_start(wr1[:], WR1)
        wvtAll = wpool.tile([128, DT, D], BF16)
        nc.scalar.dma_start(wvtAll[:], WvT.rearrange("(kd p) d -> p kd d", p=128))
        wot8All = wpool.tile([128, DT, D], FP8)
        nc.scalar.dma_start(wot8All[:], WoT8.rearrange("(kd p) d -> p kd d", p=128))
        wr2 = wpool.tile([128, 2, M2], FP8)
        nc.gpsimd.dma_start(wr2[:], WR2)

        consts = ctx.enter_context(tc.tile_pool(name="consts", bufs=1))
        gamB = consts.tile([128, D], BF16)
        nc.sync.dma_start(gamB[:], bcast(gam))
        bet2B = consts.tile([128, D], BF16)
        nc.sync.dma_start(bet2B[:], bcast(bet2))
        flagcS = consts.tile([128, 1], F32)
        nc.sync.dma_start(flagcS[:], bcast(flagc))
        scal1S = consts.tile([128, 1], F32)
        nc.sync.dma_start(scal1S[:], bcast(scal1))
        epsS = consts.tile([128, 1], F32)
        nc.vector.memset(epsS[:], LN_EPS)
        maskC = consts.tile([128, BW], BF16)
        nc.sync.dma_start(maskC[:], MaskC)
        maskB = consts.tile([128, BW], BF16)
        nc.sync.dma_start(maskB[:], MaskB)
        ScolT = consts.tile([128, DT], F32)
        nc.sync.dma_start(ScolT[:], Scol)
        ident = consts.tile([128, 128], BF16)
        make_identity(nc, ident[:])

        # ---------------- PT projection (fp8 DoubleRow) ----------------
        with tc.tile_pool(name="ps2", bufs=2, space="PSUM") as ps2:
            for m in range(DT):
                for c0 in range(0, HALF, D):
                    ps = ps2.tile([128, D], F32, tag="psp")
                    for i in range(DT // 2):
                        nc.tensor.matmul(
                            ps[:],
                            wqk8All[:, 2 * i : 2 * i + 2, ts(m, 128)],
                            at8All[:, 2 * i : 2 * i + 2, c0 : c0 + D],
                            start=(i == 0), stop=(i == DT // 2 - 1),
                            perf_mode=DR,
                        )
                    nc.scalar.copy(ptt[:, m, c0 : c0 + D], ps[:])

        # ---------------- conv1 + attention interleave ----------------
        with tc.tile_pool(name="ps1", bufs=1, space="PSUM") as ps1, \
             tc.tile_pool(name="psV", bufs=1, space="PSUM") as psV, \
             tc.tile_pool(name="psS", bufs=1, space="PSUM") as psS, \
             tc.tile_pool(name="psT", bufs=1, space="PSUM") as psT, \
             tc.tile_pool(name="psC", bufs=2, space="PSUM") as psC, \
             tc.tile_pool(name="psR", bufs=1, space="PSUM") as psR, \
             tc.tile_pool(name="smp", bufs=2) as smp, \
             tc.tile_pool(name="atp", bufs=2) as atp, \
             tc.tile_pool(name="ctp", bufs=2) as ctp, \
             tc.tile_pool(name="p3c", bufs=2) as p3c:

            def emit_conv1_chunk(c0, cw):
                for m in range(DT):
                    ps = ps1.tile([128, D], F32, tag="ps1")
                    for k in range(KT // 2):
                        q0 = c0 - 256 * k - Q1MIN
                        nc.tensor.matmul(
                            ps[:, 0:cw],
                            bsbAll[:, 2 * k : 2 * k + 2, ts(m, 128)],
                            wr1[:, :, q0 : q0 + cw],
                            start=(k == 0), stop=(k == KT // 2 - 1),
                            perf_mode=DR,
                        )
                    # balt = ps * (L/SD1) + colsum(B)[d]  (exact mean part)
                    nc.vector.tensor_scalar(
                        out=balt[:, m, c0 : c0 + cw], in0=ps[:, 0:cw],
                        scalar1=scal1S[:], scalar2=ScolT[:, m : m + 1],
                        op0=AOP.mult, op1=AOP.add,
                    )

            def emit_vrow(i):
                nr = 128 if i < NT else 8  # tile NT holds only 8 halo rows
                ps = psV.tile([128, D], F32, tag="psv")
                for kd in range(DT):
                    nc.tensor.matmul(
                        ps[0:nr, :],
                        balt[:, kd, i * 128 + 4 : i * 128 + 4 + nr],
                        wvtAll[:, kd, :],
                        start=(kd == 0), stop=(kd == DT - 1),
                    )
                nc.scalar.copy(vrows[0:nr, i, :], ps[0:nr, :])

            def emit_attn(t):
                # banded scores [128 l, 136 j] on the PE
                ps_s = psS.tile([128, BW], F32, tag="pss")
                for kd in range(DT):
                    nc.tensor.matmul(
                        ps_s[:],
                        ptt[:, kd, ts(t, 128)],
                        balt[:, kd, t * 128 + 4 : t * 128 + 4 + BW],
                        start=(kd == 0), stop=(kd == DT - 1),
                    )
                # softmax with exp(x) ~= 1+x (|s| <= ~0.04); maskC folds the
                # 1/(K1*L) descale of the raw scores
                sm = smp.tile([128, BW], BF16, tag="sm")
                nc.vector.tensor_tensor(out=sm[:], in0=ps_s[:], in1=maskC[:],
                                        op=AOP.mult)
                rs = smp.tile([128, 1], F32, tag="rs")
                nc.vector.tensor_reduce(out=rs[:], in_=sm[:],
                                        axis=mybir.AxisListType.X, op=AOP.add)
                rs9 = smp.tile([128, 1], F32, tag="rs9")
                nc.vector.tensor_scalar(out=rs9[:], in0=rs[:], scalar1=float(W9),
                                        scalar2=None, op0=AOP.add)
                racc = smp.tile([128, 1], F32, tag="racc")
                nc.vector.reciprocal(racc[:], rs9[:])
                sm1 = smp.tile([128, BW], BF16, tag="sm1")
                nc.vector.tensor_tensor(out=sm1[:], in0=sm[:], in1=maskB[:],
                                        op=AOP.add)
                attnw = smp.tile([128, BW], BF16, tag="attnw")
                nc.vector.tensor_scalar(out=attnw[:], in0=sm1[:], scalar1=racc[:],
                                        scalar2=None, op0=AOP.mult)
                # transpose the attention block
                pT1 = psT.tile([128, 128], BF16, tag="pt1")
                nc.tensor.transpose(pT1[:], attnw[:, 0:128], ident[:])
                pT2 = psT.tile([8, 128], BF16, tag="pt2")
                nc.tensor.transpose(pT2[:], attnw[:, 128:BW], ident[:])
                aT1 = atp.tile([128, 128], BF16, tag="at1")
                nc.vector.tensor_copy(aT1[:], pT1[:])
                aT2 = atp.tile([8, 128], BF16, tag="at2")
                nc.vector.tensor_copy(aT2[:], pT2[:])
                # ctx^T (feature-major) = V_rows^T @ attn^T, in fp8*CT_S
                ctile = ctp.tile([128, DT, 128], FP8, tag="ct")
                for dt_ in range(DT):
                    pc = psC.tile([128, 128], F32, tag="pc")
                    nc.tensor.matmul(pc[:], vrows[:, t, ts(dt_, 128)], aT1[:],
                                     start=True, stop=False)
                    nc.tensor.matmul(pc[:], vrows[0:8, t + 1, ts(dt_, 128)],
                                     aT2[:], start=False, stop=True)
                    nc.scalar.activation(out=ctile[:, dt_, :], in_=pc[:],
                                         func=ACTF.Copy, scale=CT_S)
                # rel = ctx @ Wo^T as fp8 DoubleRow; psr carries CT_S*K2
                psr = psR.tile([128, D], F32, tag="psrel")
                for i in range(DT // 2):
                    nc.tensor.matmul(
                        psr[:], ctile[:, 2 * i : 2 * i + 2, :],
                        wot8All[:, 2 * i : 2 * i + 2, :],
                        start=(i == 0), stop=(i == DT // 2 - 1),
                        perf_mode=DR,
                    )
                apbt = p3c.tile([128, D], F32, tag="apb")
                nc.sync.dma_start(apbt[:], Apb[ts(t, 128), :])
                h = p3c.tile([128, D], F32, tag="h")
                nc.vector.scalar_tensor_tensor(
                    out=h[:], in0=psr[:], scalar=1.0 / (CT_S * K2), in1=apbt[:],
                    op0=AOP.mult, op1=AOP.add,
                )
                st6 = p3c.tile([128, 6], F32, tag="st6")
                nc.vector.bn_stats(out=st6[:], in_=h[:])
                mv = p3c.tile([128, 2], F32, tag="mv")
                nc.vector.bn_aggr(out=mv[:], in_=st6[:])
                sdv = p3c.tile([128, 1], F32, tag="sdv")
                nc.scalar.activation(out=sdv[:], in_=mv[:, 1:2], func=ACTF.Sqrt,
                                     bias=epsS[:], scale=1.0)
                rstd = p3c.tile([128, 1], F32, tag="rstd")
                nc.vector.reciprocal(rstd[:], sdv[:])
                hn = p3c.tile([128, D], F32, tag="hn")
                nc.vector.tensor_scalar(
                    out=hn[:], in0=h[:], scalar1=mv[:, 0:1], scalar2=rstd[:],
                    op0=AOP.subtract, op1=AOP.mult,
                )
                hg = p3c.tile([128, D], F32, tag="hg")
                nc.gpsimd.tensor_tensor(out=hg[:], in0=hn[:], in1=gamB[:],
                                        op=AOP.mult)
                hb = p3c.tile([128, D], F32, tag="hb")
                nc.gpsimd.tensor_tensor(out=hb[:], in0=hg[:], in1=bet2B[:],
                                        op=AOP.add)
                aoutt = p3c.tile([128, D], F32, tag="aout")
                nc.vector.scalar_tensor_tensor(
                    out=aoutt[:], in0=apbt[:], scalar=flagcS[:], in1=hb[:],
                    op0=AOP.mult, op1=AOP.add,
                )
                nc.sync.dma_start(A_out[ts(t, 128), :], aoutt[:])
                nc.scalar.copy(aout8[:, t, :], aoutt[:])

            emit_conv1_chunk(*C1CH[0])
            emit_conv1_chunk(*C1CH[1])
            for i in range(0, 6):
                emit_vrow(i)
            for t in range(0, 4):
                emit_attn(t)
            emit_conv1_chunk(*C1CH[2])
            for i in range(6, 9):
                emit_vrow(i)
            for t in range(4, 8):
                emit_attn(t)
            emit_conv1_chunk(*C1CH[3])
            for i in range(9, 12):
                emit_vrow(i)
            for t in range(8, 11):
                emit_attn(t)
            emit_conv1_chunk(*C1CH[4])
            for i in range(12, 17):
                emit_vrow(i)
            for t in range(11, 16):
                emit_attn(t)

        # ================= conv2: partial B_out (delta part) =============
        NCH = L // D
        with tc.tile_pool(name="outp", bufs=4) as outp, \
             tc.tile_pool(name="ps4", bufs=4, space="PSUM") as ps4:
            for nch in range(NCH):
                for m in range(DT):
                    ps = ps4.tile([128, D], F32, tag="ps4")
                    for k in range(NT // 2):
                        q0 = nch * D - 256 * k - Q2MIN
                        nc.tensor.matmul(
                            ps[:], aout8[:, 2 * k : 2 * k + 2, ts(m, 128)],
                            wr2[:, :, q0 : q0 + D],
                            start=(k == 0), stop=(k == NT // 2 - 1), perf_mode=DR,
                        )
                    osb = outp.tile([128, D], F32, tag="osb")
                    nc.scalar.copy(osb[:], ps[:])
                    nc.sync.dma_start(BT_part[ts(m, 128), ts(nch, D)], osb[:])

    _split_excess_waits(nc)
    return nc


_NC_CACHE = {}


def _get_nc():
    if "nc" not in _NC_CACHE:
        _NC_CACHE["nc"] = _build_nc()
    return _NC_CACHE["nc"]


def _gate_flag(A):
    """Replicate reference _gate on host (fp64; decision margin is ~0.7)."""
    A = np.asarray(A, np.float64)
    pooled = A.mean(axis=1)
    sims = pooled @ pooled.T
    sims = sims - np.eye(sims.shape[0]) * 1e9
    srt = np.sort(sims, axis=-1)
    margin = srt[:, -1] - srt[:, -2]
    m = sims.max(axis=-1, keepdims=True)
    logp = sims - m - np.log(np.exp(sims - m).sum(axis=-1, keepdims=True))
    probs = np.exp(logp)
    entropy = -(probs * np.log(probs + 1e-9)).sum(axis=-1)
    confidence = margin - 0.5 * entropy
    fire = confidence < THRESHOLD
    return bool(fire.any())


def kernel(A, B, phase_weights, Wq, Wk, Wv, Wo, bo, ln_scale, ln_bias):
    A = np.asarray(A, np.float32)
    B = np.asarray(B, np.float32)
    phase_weights = np.asarray(phase_weights, np.float32)
    Wq, Wk, Wv, Wo = (np.asarray(x, np.float32) for x in (Wq, Wk, Wv, Wo))
    bo = np.asarray(bo, np.float32)
    ln_scale = np.asarray(ln_scale, np.float32)
    ln_bias = np.asarray(ln_bias, np.float32)

    nc = _get_nc()

    pw = phase_weights.astype(np.float64)
    wv = np.exp(pw - pw.max())
    wv = wv / wv.sum()                      # softmax weights, fp64
    ubar = 1.0 / L
    delta = wv - ubar
    dmax = max(np.abs(delta).max(), 1e-30)
    SD = 192.0 / dmax                       # scale residual into fp8 range
    d8 = (delta * SD).astype(ml_dtypes.float8_e4m3)

    flag = 1.0 if _gate_flag(A) else 0.0
    flagc = np.float32(1.0 - flag)
    gam16 = (flag * ln_scale).astype(ml_dtypes.bfloat16)
    bet2_16 = (flag * ln_bias - flagc * bo).astype(ml_dtypes.bfloat16)

    Wqk8 = ((Wq.T @ Wk) * (K1 / np.sqrt(np.float32(D)))).astype(
        ml_dtypes.float8_e4m3)
    WvT = (Wv.T / L).astype(ml_dtypes.bfloat16)
    WoT8 = (Wo.T * K2).astype(ml_dtypes.float8_e4m3)

    # band masks [128, 136]: row p attends to j in [p, p+8]
    jj = np.arange(BW)[None, :]
    pp = np.arange(128)[:, None]
    maskb = ((jj >= pp) & (jj <= pp + 2 * RADIUS))
    maskB = maskb.astype(ml_dtypes.bfloat16)
    maskC = (maskb / (K1 * L)).astype(ml_dtypes.bfloat16)

    # fp8 generator images of the delta-circulant, one per half
    p_ = np.arange(128)[:, None, None]
    i_ = np.arange(2)[None, :, None]
    wr1s, wr2s = [], []
    for h in range(2):
        own0 = h * HALF
        m1 = np.arange(M1)[None, None, :]
        idx1 = (own0 - HALO + (m1 + Q1MIN) - 128 * i_ - p_) % L
        wr1s.append(d8[idx1])
        m2 = np.arange(M2)[None, None, :]
        idx2 = (own0 + 128 * i_ + p_ - (m2 + Q2MIN)) % L
        wr2s.append(d8[idx2])

    in_maps = []
    for b in range(Bsz):
        Scol_b = B[b].sum(axis=0).astype(np.float32).reshape(DT, 128).T
        Scol_b = np.ascontiguousarray(Scol_b)
        B8 = B[b].astype(ml_dtypes.float8_e4m3)
        for h in range(2):
            own0 = h * HALF
            in_maps.append({
                "Bin8": B8,
                "WR1": wr1s[h],
                "WR2": wr2s[h],
                "AT8": np.ascontiguousarray(A[b, own0 : own0 + HALF].T).astype(
                    ml_dtypes.float8_e4m3
                ),
                "Apb": A[b, own0 : own0 + HALF] + bo,
                "Wqk8": Wqk8,
                "WvT": WvT,
                "WoT8": WoT8,
                "Scol": Scol_b,
                "MaskC": maskC,
                "MaskB": maskB,
                "gam": gam16,
                "bet2": bet2_16,
                "flagc": np.array([flagc], np.float32),
                "scal1": np.array([L / SD], np.float32),
            })

    res = run_bass_kernel_spmd(nc, in_maps, core_ids=list(range(8)))

    A_out = np.empty((Bsz, L, D), np.float32)
    B_out = np.empty((Bsz, L, D), np.float32)
    for b in range(Bsz):
        r0 = res.results[2 * b]
        r1 = res.results[2 * b + 1]
        A_out[b, :HALF] = r0["A_out"]
        A_out[b, HALF:] = r1["A_out"]
        # delta part from device; exact rank-1 mean part added on host
        B_out[b] = (r0["BT_part"] + r1["BT_part"]).T / np.float32(SD)
        B_out[b] += A_out[b].sum(axis=0, dtype=np.float64).astype(np.float32)[
            None, :] * np.float32(ubar)
    return A_out, B_out


# revision 16
# speedup vs baseline: 1.9467x; 1.0619x over previous
"""Trainium2 Bass kernel for nn_CrossResonanceLayer (sparse_attention).

Math (reference):
  w  = softmax(phase_weights)                          (L,)
  B_aligned = circconv(B, w)          = C1 @ B[b]      C1[l,m] = w[(l-m)%L]
  fire = gate(A)  -> scalar flag (host, tiny BxB math on pooled vectors)
  windowed local attention (radius 4) on (A, B_aligned), layernorm(A + rel)
  A_out = flag ? normed : A
  B_out = circconv(A_out, roll(w[::-1],1)) = C1^T @ A_out[b]

Sharding: 8 cores = (batch b in 0..3) x (sequence half h in 0..1).  Host sums
the two conv2 partials per batch; no collectives.

Implementation highlights:
 * The circulant is split C1 = (1/L)*ones + Delta.  The rank-1 mean part is
   applied as an exact per-partition scalar correction (host colsums), so
   only the small residual delta runs on the PE -- in fp8e4 DoubleRow mode
   (2 contraction rows/cycle).  Quantizing delta (2% of the weight mass,
   scaled to fp8 range) keeps conv weight error ~50x below quantizing w.
 * conv1/conv2 moving operands are slices of a single SBUF-resident
   [128, 2, M] generator image of delta; no circulant HBM traffic at all.
 * The q/o projections also run as fp8 DoubleRow matmuls with build-time
   power-of-two scale folding (K1/K2), descaled for free in later ops.
 * Attention is PE-centric: banded [128 l, 136 j] score block per tile,
   row-major softmax with exp(x)~=1+x and per-partition scalars, PE
   transpose of the attention block, ctx^T = V_rows^T @ attn^T directly in
   feature-major layout for the Wo matmul.
 * Elementwise work is spread across DVE / ACT / GPSIMD; large input DMAs
   are issued from otherwise-idle engine queues in need-order.
"""
import sys

sys.path.insert(0, "/opt/trn_rl_repo")

from contextlib import ExitStack

import numpy as np
import ml_dtypes

import concourse.bass as bass
import concourse.tile as tile
from concourse import mybir
from concourse.bass_utils import run_bass_kernel_spmd
from concourse.masks import make_identity

F32 = mybir.dt.float32
BF16 = mybir.dt.bfloat16
FP8 = mybir.dt.float8e4
AOP = mybir.AluOpType
ACTF = mybir.ActivationFunctionType
DR = mybir.MatmulPerfMode.DoubleRow

Bsz, L, D = 4, 4096, 512
HALF = L // 2              # 2048 rows per core
HALO = 8                   # windowed attention needs only +-4
WID = HALF + 2 * HALO      # 2064 halo-extended rows
NT = HALF // 128           # 16 own l-tiles
KT = L // 128              # 32 k-tiles along L
DT = D // 128              # 4 d-tiles
RADIUS = 4
W9 = 2 * RADIUS + 1        # 9
BW = 128 + 2 * RADIUS      # 136 banded score width
LN_EPS = 1e-5
THRESHOLD = 0.15

Q1MIN = -(L - 256)         # -3840 (conv1 contracts over L: 16 k-pairs)
M1 = WID - Q1MIN           # 5904
Q2MIN = -(HALF - 256)      # -1792 (conv2 contracts over HALF: 8 k-pairs)
M2 = L - Q2MIN             # 5888

K1 = 256.0                 # build-time fp8 scale for Wqk
K2 = 64.0                  # build-time fp8 scale for WoT
CT_S = 64.0                # build-time fp8 scale for ctx tiles
# conv1 chunk widths (<=512 for one PSUM bank, no tiny tail chunk)
C1CH = [(0, 416), (416, 416), (832, 416), (1248, 416), (1664, 400)]


def _split_excess_waits(nc, max_waits=1):
    """This walrus build accepts at most one sem-wait command per instruction.
    Move excess waits onto same-engine NOPs placed right before the owner."""
    ctr = 0
    for fn in nc.m.functions:
        for bb in fn.blocks:
            out = []
            changed = False
            for inst in bb.instructions:
                si = inst.sync_info
                if si is not None and len(si.on_wait) > max_waits:
                    waits = list(si.on_wait)
                    keep = waits[-max_waits:]
                    extra = waits[:-max_waits]
                    for i in range(0, len(extra), max_waits):
                        nop = mybir.InstNoOp(name=f"waitsplit-{ctr}")
                        ctr += 1
                        nop.engine = inst.engine
                        nop.sync_info = mybir.SyncInfo(
                            on_wait=extra[i : i + max_waits], on_update=[]
                        )
                        out.append(nop)
                    si.on_wait = keep
                    changed = True
                out.append(inst)
            if changed:
                bb.instructions = out
    return ctr


def _build_nc():
    nc = bass.Bass("TRN2", target_bir_lowering=False, debug=False, num_devices=8)

    # ---- inputs (per core) ----
    Bin8 = nc.dram_tensor("Bin8", [L, D], FP8, kind="ExternalInput").ap()
    WR1 = nc.dram_tensor("WR1", [128, 2, M1], FP8, kind="ExternalInput").ap()
    WR2 = nc.dram_tensor("WR2", [128, 2, M2], FP8, kind="ExternalInput").ap()
    AT8 = nc.dram_tensor("AT8", [D, HALF], FP8, kind="ExternalInput").ap()
    Apb = nc.dram_tensor("Apb", [HALF, D], F32, kind="ExternalInput").ap()  # A + bo
    Wqk8 = nc.dram_tensor("Wqk8", [D, D], FP8, kind="ExternalInput").ap()  # K1*WqT@Wk/sqrt(d)
    WvT = nc.dram_tensor("WvT", [D, D], BF16, kind="ExternalInput").ap()   # Wv.T/L
    WoT8 = nc.dram_tensor("WoT8", [D, D], FP8, kind="ExternalInput").ap()  # K2*Wo.T
    MaskC = nc.dram_tensor("MaskC", [128, BW], BF16, kind="ExternalInput").ap()
    MaskB = nc.dram_tensor("MaskB", [128, BW], BF16, kind="ExternalInput").ap()
    gam = nc.dram_tensor("gam", [D], BF16, kind="ExternalInput").ap()   # flag*ln_scale
    bet2 = nc.dram_tensor("bet2", [D], BF16, kind="ExternalInput").ap() # flag*ln_bias-(1-flag)*bo
    flagc = nc.dram_tensor("flagc", [1], F32, kind="ExternalInput").ap()  # 1-flag
    scal1 = nc.dram_tensor("scal1", [1], F32, kind="ExternalInput").ap()  # L/SD1

    # ---- outputs ----
    A_out = nc.dram_tensor("A_out", [HALF, D], F32, kind="ExternalOutput").ap()
    BT_part = nc.dram_tensor("BT_part", [D, L], F32, kind="ExternalOutput").ap()

    def bcast(row_ap, parts=128):
        return bass.AP(
            tensor=row_ap.tensor,
            offset=row_ap.offset,
            ap=[[0, parts]] + list(row_ap.ap),
        )

    ts = bass.ts

    with tile.TileContext(nc) as tc, ExitStack() as ctx:
        # persistent activations
        persist = ctx.enter_context(tc.tile_pool(name="persist", bufs=1))
        ptt = persist.tile([128, DT, HALF], BF16)   # K1*4096*(A Wqk)^T feature-major
        balt = persist.tile([128, DT, WID], BF16)   # 4096*B_al^T feature-major
        vrows = persist.tile([128, NT + 1, D], BF16)  # V rows, shifted by -4
        aout8 = persist.tile([128, NT, D], FP8)     # A_out quantized for conv2

        wpool = ctx.enter_context(tc.tile_pool(name="wpool", bufs=1))
        # issue big loads from idle engine queues, in need-order
        wqk8All = wpool.tile([128, DT, D], FP8)
        nc.sync.dma_start(wqk8All[:], Wqk8.rearrange("(kd p) d -> p kd d", p=128))
        at8All = wpool.tile([128, DT, HALF], FP8)
        nc.sync.dma_start(at8All[:], AT8.rearrange("(kd p) l -> p kd l", p=128))
        bsbAll = wpool.tile([128, KT, D], FP8)
        nc.gpsimd.dma_start(bsbAll[:], Bin8.rearrange("(kt p) d -> p kt d", p=128))
        wr1 = wpool.tile([128, 2, M1], FP8)
        nc.scalar.dma_start(wr1[:], WR1)
        wvtAll = wpool.tile([128, DT, D], BF16)
        nc.scalar.dma_start(wvtAll[:], WvT.rearrange("(kd p) d -> p kd d", p=128))
        wot8All = wpool.tile([128, DT, D], FP8)
        nc.scalar.dma_start(wot8All[:], WoT8.rearrange("(kd p) d -> p kd d", p=128))
        wr2 = wpool.tile([128, 2, M2], FP8)
        nc.gpsimd.dma_start(wr2[:], WR2)

        consts = ctx.enter_context(tc.tile_pool(name="consts", bufs=1))
        gamB = consts.tile([128, D], BF16)
        nc.sync.dma_start(gamB[:], bcast(gam))
        bet2B = consts.tile([128, D], BF16)
        nc.sync.dma_start(bet2B[:], bcast(bet2))
        flagcS = consts.tile([128, 1], F32)
        nc.sync.dma_start(flagcS[:], bcast(flagc))
        scal1S = consts.tile([128, 1], F32)
        nc.sync.dma_start(scal1S[:], bcast(scal1))
        epsS = consts.tile([128, 1], F32)
        nc.vector.memset(epsS[:], LN_EPS)
        maskC = consts.tile([128, BW], BF16)
        nc.sync.dma_start(maskC[:], MaskC)
        maskB = consts.tile([128, BW], BF16)
        nc.sync.dma_start(maskB[:], MaskB)
        ident = consts.tile([128, 128], BF16)
        make_identity(nc, ident[:])

        # ---------------- PT projection (fp8 DoubleRow) ----------------
        with tc.tile_pool(name="ps2", bufs=2, space="PSUM") as ps2:
            for m in range(DT):
                for c0 in range(0, HALF, D):
                    ps = ps2.tile([128, D], F32, tag="psp")
                    for i in range(DT // 2):
                        nc.tensor.matmul(
                            ps[:],
                            wqk8All[:, 2 * i : 2 * i + 2, ts(m, 128)],
                            at8All[:, 2 * i : 2 * i + 2, c0 : c0 + D],
                            start=(i == 0), stop=(i == DT // 2 - 1),
                            perf_mode=DR,
                        )
                    nc.scalar.copy(ptt[:, m, c0 : c0 + D], ps[:])

        # ---------------- conv1 + attention interleave ----------------
        with tc.tile_pool(name="ps1", bufs=1, space="PSUM") as ps1, \
             tc.tile_pool(name="psV", bufs=1, space="PSUM") as psV, \
             tc.tile_pool(name="psS", bufs=1, space="PSUM") as psS, \
             tc.tile_pool(name="psT", bufs=1, space="PSUM") as psT, \
             tc.tile_pool(name="psC", bufs=2, space="PSUM") as psC, \
             tc.tile_pool(name="psR", bufs=1, space="PSUM") as psR, \
             tc.tile_pool(name="smp", bufs=2) as smp, \
             tc.tile_pool(name="atp", bufs=2) as atp, \
             tc.tile_pool(name="ctp", bufs=2) as ctp, \
             tc.tile_pool(name="p3c", bufs=2) as p3c:

            def emit_conv1_chunk(c0, cw):
                for m in range(DT):
                    ps = ps1.tile([128, D], F32, tag="ps1")
                    for k in range(KT // 2):
                        q0 = c0 - 256 * k - Q1MIN
                        nc.tensor.matmul(
                            ps[:, 0:cw],
                            bsbAll[:, 2 * k : 2 * k + 2, ts(m, 128)],
                            wr1[:, :, q0 : q0 + cw],
                            start=(k == 0), stop=(k == KT // 2 - 1),
                            perf_mode=DR,
                        )
                    # balt = mean-removed B_al * L (pure delta part; the rank-1
                    # mean of B_al is folded into Apb/bet2 on the host)
                    nc.vector.tensor_scalar(
                        out=balt[:, m, c0 : c0 + cw], in0=ps[:, 0:cw],
                        scalar1=scal1S[:], scalar2=None, op0=AOP.mult,
                    )

            def emit_vrow(i):
                nr = 128 if i < NT else 8  # tile NT holds only 8 halo rows
                ps = psV.tile([128, D], F32, tag="psv")
                for kd in range(DT):
                    nc.tensor.matmul(
                        ps[0:nr, :],
                        balt[:, kd, i * 128 + 4 : i * 128 + 4 + nr],
                        wvtAll[:, kd, :],
                        start=(kd == 0), stop=(kd == DT - 1),
                    )
                nc.scalar.copy(vrows[0:nr, i, :], ps[0:nr, :])

            def emit_attn(t):
                # banded scores [128 l, 136 j] on the PE
                ps_s = psS.tile([128, BW], F32, tag="pss")
                for kd in range(DT):
                    nc.tensor.matmul(
                        ps_s[:],
                        ptt[:, kd, ts(t, 128)],
                        balt[:, kd, t * 128 + 4 : t * 128 + 4 + BW],
                        start=(kd == 0), stop=(kd == DT - 1),
                    )
                # softmax with exp(x) ~= 1+x (|s| <= ~0.04); maskC folds the
                # 1/(K1*L) descale of the raw scores
                sm = smp.tile([128, BW], BF16, tag="sm")
                nc.vector.tensor_tensor(out=sm[:], in0=ps_s[:], in1=maskC[:],
                                        op=AOP.mult)
                rs = smp.tile([128, 1], F32, tag="rs")
                nc.vector.tensor_reduce(out=rs[:], in_=sm[:],
                                        axis=mybir.AxisListType.X, op=AOP.add)
                rs9 = smp.tile([128, 1], F32, tag="rs9")
                nc.vector.tensor_scalar(out=rs9[:], in0=rs[:], scalar1=float(W9),
                                        scalar2=None, op0=AOP.add)
                racc = smp.tile([128, 1], F32, tag="racc")
                nc.vector.reciprocal(racc[:], rs9[:])
                sm1 = smp.tile([128, BW], BF16, tag="sm1")
                nc.vector.tensor_tensor(out=sm1[:], in0=sm[:], in1=maskB[:],
                                        op=AOP.add)
                attnw = smp.tile([128, BW], BF16, tag="attnw")
                nc.vector.tensor_scalar(out=attnw[:], in0=sm1[:], scalar1=racc[:],
                                        scalar2=None, op0=AOP.mult)
                # transpose the attention block
                pT1 = psT.tile([128, 128], BF16, tag="pt1")
                nc.tensor.transpose(pT1[:], attnw[:, 0:128], ident[:])
                pT2 = psT.tile([8, 128], BF16, tag="pt2")
                nc.tensor.transpose(pT2[:], attnw[:, 128:BW], ident[:])
                aT1 = atp.tile([128, 128], BF16, tag="at1")
                nc.vector.tensor_copy(aT1[:], pT1[:])
                aT2 = atp.tile([8, 128], BF16, tag="at2")
                nc.vector.tensor_copy(aT2[:], pT2[:])
                # ctx^T (feature-major) = V_rows^T @ attn^T, in fp8*CT_S
                ctile = ctp.tile([128, DT, 128], FP8, tag="ct")
                for dt_ in range(DT):
                    pc = psC.tile([128, 128], F32, tag="pc")
                    nc.tensor.matmul(pc[:], vrows[:, t, ts(dt_, 128)], aT1[:],
                                     start=True, stop=False)
                    nc.tensor.matmul(pc[:], vrows[0:8, t + 1, ts(dt_, 128)],
                                     aT2[:], start=False, stop=True)
                    nc.scalar.activation(out=ctile[:, dt_, :], in_=pc[:],
                                         func=ACTF.Copy, scale=CT_S)
                # rel = ctx @ Wo^T as fp8 DoubleRow; psr carries CT_S*K2
                psr = psR.tile([128, D], F32, tag="psrel")
                for i in range(DT // 2):
                    nc.tensor.matmul(
                        psr[:], ctile[:, 2 * i : 2 * i + 2, :],
                        wot8All[:, 2 * i : 2 * i + 2, :],
                        start=(i == 0), stop=(i == DT // 2 - 1),
                        perf_mode=DR,
                    )
                apbt = p3c.tile([128, D], F32, tag="apb")
                nc.sync.dma_start(apbt[:], Apb[ts(t, 128), :])
                h = p3c.tile([128, D], F32, tag="h")
                nc.vector.scalar_tensor_tensor(
                    out=h[:], in0=psr[:], scalar=1.0 / (CT_S * K2), in1=apbt[:],
                    op0=AOP.mult, op1=AOP.add,
                )
                st6 = p3c.tile([128, 6], F32, tag="st6")
                nc.vector.bn_stats(out=st6[:], in_=h[:])
                mv = p3c.tile([128, 2], F32, tag="mv")
                nc.vector.bn_aggr(out=mv[:], in_=st6[:])
                sdv = p3c.tile([128, 1], F32, tag="sdv")
                nc.scalar.activation(out=sdv[:], in_=mv[:, 1:2], func=ACTF.Sqrt,
                                     bias=epsS[:], scale=1.0)
                rstd = p3c.tile([128, 1], F32, tag="rstd")
                nc.vector.reciprocal(rstd[:], sdv[:])
                hn = p3c.tile([128, D], F32, tag="hn")
                nc.vector.tensor_scalar(
                    out=hn[:], in0=h[:], scalar1=mv[:, 0:1], scalar2=rstd[:],
                    op0=AOP.subtract, op1=AOP.mult,
                )
                hg = p3c.tile([128, D], F32, tag="hg")
                nc.gpsimd.tensor_tensor(out=hg[:], in0=hn[:], in1=gamB[:],
                                        op=AOP.mult)
                hb = p3c.tile([128, D], F32, tag="hb")
                nc.gpsimd.tensor_tensor(out=hb[:], in0=hg[:], in1=bet2B[:],
                                        op=AOP.add)
                aoutt = p3c.tile([128, D], F32, tag="aout")
                nc.vector.scalar_tensor_tensor(
                    out=aoutt[:], in0=apbt[:], scalar=flagcS[:], in1=hb[:],
                    op0=AOP.mult, op1=AOP.add,
                )
                nc.sync.dma_start(A_out[ts(t, 128), :], aoutt[:])
                nc.scalar.copy(aout8[:, t, :], aoutt[:])

            emit_conv1_chunk(*C1CH[0])
            emit_conv1_chunk(*C1CH[1])
            for i in range(0, 6):
                emit_vrow(i)
            for t in range(0, 4):
                emit_attn(t)
            emit_conv1_chunk(*C1CH[2])
            for i in range(6, 9):
                emit_vrow(i)
            for t in range(4, 8):
                emit_attn(t)
            emit_conv1_chunk(*C1CH[3])
            for i in range(9, 12):
                emit_vrow(i)
            for t in range(8, 11):
                emit_attn(t)
            emit_conv1_chunk(*C1CH[4])
            for i in range(12, 17):
                emit_vrow(i)
            for t in range(11, 16):
                emit_attn(t)

        # ================= conv2: partial B_out (delta part) =============
        NCH = L // D
        with tc.tile_pool(name="outp", bufs=4) as outp, \
             tc.tile_pool(name="ps4", bufs=4, space="PSUM") as ps4:
            for nch in range(NCH):
                for m in range(DT):
                    ps = ps4.tile([128, D], F32, tag="ps4")
                    for k in range(NT // 2):
                        q0 = nch * D - 256 * k - Q2MIN
                        nc.tensor.matmul(
                            ps[:], aout8[:, 2 * k : 2 * k + 2, ts(m, 128)],
                            wr2[:, :, q0 : q0 + D],
                            start=(k == 0), stop=(k == NT // 2 - 1), perf_mode=DR,
                        )
                    osb = outp.tile([128, D], F32, tag="osb")
                    nc.scalar.copy(osb[:], ps[:])
                    nc.sync.dma_start(BT_part[ts(m, 128), ts(nch, D)], osb[:])

    _split_excess_waits(nc)
    return nc


_NC_CACHE = {}


def _get_nc():
    if "nc" not in _NC_CACHE:
        _NC_CACHE["nc"] = _build_nc()
    return _NC_CACHE["nc"]


def _gate_flag(A):
    """Replicate reference _gate on host (fp64; decision margin is ~0.7)."""
    A = np.asarray(A, np.float64)
    pooled = A.mean(axis=1)
    sims = pooled @ pooled.T
    sims = sims - np.eye(sims.shape[0]) * 1e9
    srt = np.sort(sims, axis=-1)
    margin = srt[:, -1] - srt[:, -2]
    m = sims.max(axis=-1, keepdims=True)
    logp = sims - m - np.log(np.exp(sims - m).sum(axis=-1, keepdims=True))
    probs = np.exp(logp)
    entropy = -(probs * np.log(probs + 1e-9)).sum(axis=-1)
    confidence = margin - 0.5 * entropy
    fire = confidence < THRESHOLD
    return bool(fire.any())


def kernel(A, B, phase_weights, Wq, Wk, Wv, Wo, bo, ln_scale, ln_bias):
    A = np.asarray(A, np.float32)
    B = np.asarray(B, np.float32)
    phase_weights = np.asarray(phase_weights, np.float32)
    Wq, Wk, Wv, Wo = (np.asarray(x, np.float32) for x in (Wq, Wk, Wv, Wo))
    bo = np.asarray(bo, np.float32)
    ln_scale = np.asarray(ln_scale, np.float32)
    ln_bias = np.asarray(ln_bias, np.float32)

    nc = _get_nc()

    pw = phase_weights.astype(np.float64)
    wv = np.exp(pw - pw.max())
    wv = wv / wv.sum()                      # softmax weights, fp64
    ubar = 1.0 / L
    delta = wv - ubar
    dmax = max(np.abs(delta).max(), 1e-30)
    SD = 192.0 / dmax                       # scale residual into fp8 range
    d8 = (delta * SD).astype(ml_dtypes.float8_e4m3)

    flag = 1.0 if _gate_flag(A) else 0.0
    flagc = np.float32(1.0 - flag)
    gam16 = (flag * ln_scale).astype(ml_dtypes.bfloat16)

    Wqk8 = ((Wq.T @ Wk) * (K1 / np.sqrt(np.float32(D)))).astype(
        ml_dtypes.float8_e4m3)
    WvT = (Wv.T / L).astype(ml_dtypes.bfloat16)
    WoT8 = (Wo.T * K2).astype(ml_dtypes.float8_e4m3)

    # band masks [128, 136]: row p attends to j in [p, p+8]
    jj = np.arange(BW)[None, :]
    pp = np.arange(128)[:, None]
    maskb = ((jj >= pp) & (jj <= pp + 2 * RADIUS))
    maskB = maskb.astype(ml_dtypes.bfloat16)
    maskC = (maskb / (K1 * L)).astype(ml_dtypes.bfloat16)

    # fp8 generator images of the delta-circulant, one per half
    p_ = np.arange(128)[:, None, None]
    i_ = np.arange(2)[None, :, None]
    wr1s, wr2s = [], []
    for h in range(2):
        own0 = h * HALF
        m1 = np.arange(M1)[None, None, :]
        idx1 = (own0 - HALO + (m1 + Q1MIN) - 128 * i_ - p_) % L
        wr1s.append(d8[idx1])
        m2 = np.arange(M2)[None, None, :]
        idx2 = (own0 + 128 * i_ + p_ - (m2 + Q2MIN)) % L
        wr2s.append(d8[idx2])

    in_maps = []
    for b in range(Bsz):
        # exact rank-1 parts: attention on mean-removed V; the constant
        # vbar @ Wo.T rides along with A + bo (and is cancelled by bet2
        # in the no-fire path so A_out == A stays exact)
        vbar = (B[b].sum(axis=0, dtype=np.float64) / L) @ Wv.T.astype(np.float64)
        rel_const = (vbar @ Wo.T.astype(np.float64)).astype(np.float32)
        bet2_16 = (flag * ln_bias - flagc * (bo + rel_const)).astype(
            ml_dtypes.bfloat16)
        B8 = B[b].astype(ml_dtypes.float8_e4m3)
        for h in range(2):
            own0 = h * HALF
            in_maps.append({
                "Bin8": B8,
                "WR1": wr1s[h],
                "WR2": wr2s[h],
                "AT8": np.ascontiguousarray(A[b, own0 : own0 + HALF].T).astype(
                    ml_dtypes.float8_e4m3
                ),
                "Apb": A[b, own0 : own0 + HALF] + bo + rel_const[None, :],
                "Wqk8": Wqk8,
                "WvT": WvT,
                "WoT8": WoT8,
                "MaskC": maskC,
                "MaskB": maskB,
                "gam": gam16,
                "bet2": bet2_16,
                "flagc": np.array([flagc], np.float32),
                "scal1": np.array([L / SD], np.float32),
            })

    res = run_bass_kernel_spmd(nc, in_maps, core_ids=list(range(8)))

    A_out = np.empty((Bsz, L, D), np.float32)
    B_out = np.empty((Bsz, L, D), np.float32)
    for b in range(Bsz):
        r0 = res.results[2 * b]
        r1 = res.results[2 * b + 1]
        A_out[b, :HALF] = r0["A_out"]
        A_out[b, HALF:] = r1["A_out"]
        # delta part from device; exact rank-1 mean part added on host
        B_out[b] = (r0["BT_part"] + r1["BT_part"]).T / np.float32(SD)
        B_out[b] += A_out[b].sum(axis=0, dtype=np.float64).astype(np.float32)[
            None, :] * np.float32(ubar)
    return A_out, B_out


# revision 24
# speedup vs baseline: 1.9603x; 1.0070x over previous
"""Trainium2 Bass kernel for nn_CrossResonanceLayer (sparse_attention).

Math (reference):
  w  = softmax(phase_weights)                          (L,)
  B_aligned = circconv(B, w)          = C1 @ B[b]      C1[l,m] = w[(l-m)%L]
  fire = gate(A)  -> scalar flag (host, tiny BxB math on pooled vectors)
  windowed local attention (radius 4) on (A, B_aligned), layernorm(A + rel)
  A_out = flag ? normed : A
  B_out = circconv(A_out, roll(w[::-1],1)) = C1^T @ A_out[b]

Sharding: 8 cores = (batch b in 0..3) x (sequence half h in 0..1).  Host sums
the two conv2 partials per batch; no collectives.

Implementation highlights:
 * The circulant is split C1 = (1/L)*ones + Delta.  The rank-1 mean part is
   applied as an exact per-partition scalar correction (host colsums), so
   only the small residual delta runs on the PE -- in fp8e4 DoubleRow mode
   (2 contraction rows/cycle).  Quantizing delta (2% of the weight mass,
   scaled to fp8 range) keeps conv weight error ~50x below quantizing w.
 * conv1/conv2 moving operands are slices of a single SBUF-resident
   [128, 2, M] generator image of delta; no circulant HBM traffic at all.
 * The q/o projections also run as fp8 DoubleRow matmuls with build-time
   power-of-two scale folding (K1/K2), descaled for free in later ops.
 * Attention is PE-centric: banded [128 l, 136 j] score block per tile,
   row-major softmax with exp(x)~=1+x and per-partition scalars, PE
   transpose of the attention block, ctx^T = V_rows^T @ attn^T directly in
   feature-major layout for the Wo matmul.
 * Elementwise work is spread across DVE / ACT / GPSIMD; large input DMAs
   are issued from otherwise-idle engine queues in need-order.
"""
import sys

sys.path.insert(0, "/opt/trn_rl_repo")

from contextlib import ExitStack

import numpy as np
import ml_dtypes

import concourse.bass as bass
import concourse.tile as tile
from concourse import mybir
from concourse.bass_utils import run_bass_kernel_spmd
from concourse.masks import make_identity

F32 = mybir.dt.float32
BF16 = mybir.dt.bfloat16
FP8 = mybir.dt.float8e4
AOP = mybir.AluOpType
ACTF = mybir.ActivationFunctionType
DR = mybir.MatmulPerfMode.DoubleRow

Bsz, L, D = 4, 4096, 512
HALF = L // 2              # 2048 rows per core
HALO = 8                   # windowed attention needs only +-4
WID = HALF + 2 * HALO      # 2064 halo-extended rows
NT = HALF // 128           # 16 own l-tiles
KT = L // 128              # 32 k-tiles along L
DT = D // 128              # 4 d-tiles
RADIUS = 4
W9 = 2 * RADIUS + 1        # 9
BW = 128 + 2 * RADIUS      # 136 banded score width
LN_EPS = 1e-5
THRESHOLD = 0.15

Q1MIN = -(L - 256)         # -3840 (conv1 contracts over L: 16 k-pairs)
M1 = WID - Q1MIN           # 5904
Q2MIN = -(HALF - 256)      # -1792 (conv2 contracts over HALF: 8 k-pairs)
M2 = L - Q2MIN             # 5888

K1 = 256.0                 # build-time fp8 scale for Wqk
K2 = 64.0                  # build-time fp8 scale for WoT
CT_S = 64.0                # build-time fp8 scale for ctx tiles
# conv1 chunk widths (<=512 for one PSUM bank; small last chunk so most
# attention tiles can be emitted before the final chunk)
C1CH = [(0, 496), (496, 496), (992, 496), (1488, 496), (1984, 80)]


def _split_excess_waits(nc, max_waits=1):
    """This walrus build accepts at most one sem-wait command per instruction.
    Move excess waits onto same-engine NOPs placed right before the owner."""
    ctr = 0
    for fn in nc.m.functions:
        for bb in fn.blocks:
            out = []
            changed = False
            for inst in bb.instructions:
                si = inst.sync_info
                if si is not None and len(si.on_wait) > max_waits:
                    waits = list(si.on_wait)
                    keep = waits[-max_waits:]
                    extra = waits[:-max_waits]
                    for i in range(0, len(extra), max_waits):
                        nop = mybir.InstNoOp(name=f"waitsplit-{ctr}")
                        ctr += 1
                        nop.engine = inst.engine
                        nop.sync_info = mybir.SyncInfo(
                            on_wait=extra[i : i + max_waits], on_update=[]
                        )
                        out.append(nop)
                    si.on_wait = keep
                    changed = True
                out.append(inst)
            if changed:
                bb.instructions = out
    return ctr


def _build_nc():
    nc = bass.Bass("TRN2", target_bir_lowering=False, debug=False, num_devices=8)

    # ---- inputs (per core) ----
    Bin8 = nc.dram_tensor("Bin8", [L, D], FP8, kind="ExternalInput").ap()
    WR1 = nc.dram_tensor("WR1", [128, 2, M1], FP8, kind="ExternalInput").ap()
    WR2 = nc.dram_tensor("WR2", [128, 2, M2], FP8, kind="ExternalInput").ap()
    AT8 = nc.dram_tensor("AT8", [D, HALF], FP8, kind="ExternalInput").ap()
    Apb = nc.dram_tensor("Apb", [HALF, D], F32, kind="ExternalInput").ap()  # A + bo
    Wqk8 = nc.dram_tensor("Wqk8", [D, D], FP8, kind="ExternalInput").ap()  # K1*WqT@Wk/sqrt(d)
    WvT = nc.dram_tensor("WvT", [D, D], BF16, kind="ExternalInput").ap()   # Wv.T/L
    WoT8 = nc.dram_tensor("WoT8", [D, D], FP8, kind="ExternalInput").ap()  # K2*Wo.T
    MaskC = nc.dram_tensor("MaskC", [128, BW], BF16, kind="ExternalInput").ap()
    MaskB = nc.dram_tensor("MaskB", [128, BW], BF16, kind="ExternalInput").ap()
    gam = nc.dram_tensor("gam", [D], BF16, kind="ExternalInput").ap()   # flag*ln_scale
    bet2 = nc.dram_tensor("bet2", [D], BF16, kind="ExternalInput").ap() # flag*ln_bias-(1-flag)*bo
    flagc = nc.dram_tensor("flagc", [1], F32, kind="ExternalInput").ap()  # 1-flag
    scal1 = nc.dram_tensor("scal1", [1], F32, kind="ExternalInput").ap()  # L/SD1

    # ---- outputs ----
    A_out = nc.dram_tensor("A_out", [HALF, D], F32, kind="ExternalOutput").ap()
    BT_part = nc.dram_tensor("BT_part", [D, L], F32, kind="ExternalOutput").ap()

    def bcast(row_ap, parts=128):
        return bass.AP(
            tensor=row_ap.tensor,
            offset=row_ap.offset,
            ap=[[0, parts]] + list(row_ap.ap),
        )

    ts = bass.ts

    with tile.TileContext(nc) as tc, ExitStack() as ctx:
        # persistent activations
        persist = ctx.enter_context(tc.tile_pool(name="persist", bufs=1))
        ptt = persist.tile([128, DT, HALF], BF16)   # K1*4096*(A Wqk)^T feature-major
        balt = persist.tile([128, DT, WID], BF16)   # 4096*B_al^T feature-major
        vrows = persist.tile([128, NT + 1, D], BF16)  # V rows, shifted by -4
        aout8 = persist.tile([128, NT, D], FP8)     # A_out quantized for conv2

        wpool = ctx.enter_context(tc.tile_pool(name="wpool", bufs=1))
        # issue big loads from idle engine queues, in need-order
        wqk8All = wpool.tile([128, DT, D], FP8)
        nc.sync.dma_start(wqk8All[:], Wqk8.rearrange("(kd p) d -> p kd d", p=128))
        # at8 split into column blocks across three DMA queues so PT (which
        # runs c0-outer) can start as soon as the first block lands
        at8All = wpool.tile([128, DT, HALF], FP8)
        at8r = AT8.rearrange("(kd p) l -> p kd l", p=128)
        at8q = [nc.sync, nc.scalar, nc.gpsimd, nc.sync]
        for c in range(DT):
            at8q[c].dma_start(at8All[:, :, ts(c, D)], at8r[:, :, ts(c, D)])
        bsbAll = wpool.tile([128, KT, D], FP8)
        nc.gpsimd.dma_start(bsbAll[:], Bin8.rearrange("(kt p) d -> p kt d", p=128))
        wr1 = wpool.tile([128, 2, M1], FP8)
        nc.scalar.dma_start(wr1[:], WR1)
        wvtAll = wpool.tile([128, DT, D], BF16)
        nc.scalar.dma_start(wvtAll[:], WvT.rearrange("(kd p) d -> p kd d", p=128))
        wot8All = wpool.tile([128, DT, D], FP8)
        nc.scalar.dma_start(wot8All[:], WoT8.rearrange("(kd p) d -> p kd d", p=128))
        wr2 = wpool.tile([128, 2, M2], FP8)
        nc.gpsimd.dma_start(wr2[:], WR2)

        consts = ctx.enter_context(tc.tile_pool(name="consts", bufs=1))
        gamB = consts.tile([128, D], BF16)
        nc.sync.dma_start(gamB[:], bcast(gam))
        bet2B = consts.tile([128, D], BF16)
        nc.sync.dma_start(bet2B[:], bcast(bet2))
        flagcS = consts.tile([128, 1], F32)
        nc.sync.dma_start(flagcS[:], bcast(flagc))
        scal1S = consts.tile([128, 1], F32)
        nc.sync.dma_start(scal1S[:], bcast(scal1))
        epsS = consts.tile([128, 1], F32)
        nc.vector.memset(epsS[:], LN_EPS)
        maskC = consts.tile([128, BW], BF16)
        nc.sync.dma_start(maskC[:], MaskC)
        maskB = consts.tile([128, BW], BF16)
        nc.sync.dma_start(maskB[:], MaskB)
        ident = consts.tile([128, 128], BF16)
        make_identity(nc, ident[:])

        # ---------------- PT projection (fp8 DoubleRow) ----------------
        with tc.tile_pool(name="ps2", bufs=2, space="PSUM") as ps2:
            for c0 in range(0, HALF, D):
                for m in range(DT):
                    ps = ps2.tile([128, D], F32, tag="psp")
                    for i in range(DT // 2):
                        nc.tensor.matmul(
                            ps[:],
                            wqk8All[:, 2 * i : 2 * i + 2, ts(m, 128)],
                            at8All[:, 2 * i : 2 * i + 2, c0 : c0 + D],
                            start=(i == 0), stop=(i == DT // 2 - 1),
                            perf_mode=DR,
                        )
                    nc.scalar.copy(ptt[:, m, c0 : c0 + D], ps[:])

        # ---------------- conv1 + attention interleave ----------------
        with tc.tile_pool(name="ps1", bufs=2, space="PSUM") as ps1, \
             tc.tile_pool(name="psV", bufs=2, space="PSUM") as psV, \
             tc.tile_pool(name="psS", bufs=1, space="PSUM") as psS, \
             tc.tile_pool(name="psC", bufs=1, space="PSUM") as psC, \
             tc.tile_pool(name="psR", bufs=1, space="PSUM") as psR, \
             tc.tile_pool(name="smp", bufs=2) as smp, \
             tc.tile_pool(name="atp", bufs=2) as atp, \
             tc.tile_pool(name="ctp", bufs=2) as ctp, \
             tc.tile_pool(name="p3c", bufs=2) as p3c:

            def emit_conv1_chunk(c0, cw):
                for m in range(DT):
                    ps = ps1.tile([128, D], F32, tag="ps1")
                    for k in range(KT // 2):
                        q0 = c0 - 256 * k - Q1MIN
                        nc.tensor.matmul(
                            ps[:, 0:cw],
                            bsbAll[:, 2 * k : 2 * k + 2, ts(m, 128)],
                            wr1[:, :, q0 : q0 + cw],
                            start=(k == 0), stop=(k == KT // 2 - 1),
                            perf_mode=DR,
                        )
                    # balt = mean-removed B_al * L (pure delta part; the rank-1
                    # mean of B_al is folded into Apb/bet2 on the host)
                    nc.vector.tensor_scalar(
                        out=balt[:, m, c0 : c0 + cw], in0=ps[:, 0:cw],
                        scalar1=scal1S[:], scalar2=None, op0=AOP.mult,
                    )

            def emit_vrow(i):
                nr = 128 if i < NT else 8  # tile NT holds only 8 halo rows
                ps = psV.tile([128, D], F32, tag="psv")
                for kd in range(DT):
                    nc.tensor.matmul(
                        ps[0:nr, :],
                        balt[:, kd, i * 128 + 4 : i * 128 + 4 + nr],
                        wvtAll[:, kd, :],
                        start=(kd == 0), stop=(kd == DT - 1),
                    )
                nc.scalar.copy(vrows[0:nr, i, :], ps[0:nr, :])

            def emit_attn(t):
                # banded scores [128 l, 136 j] on the PE
                ps_s = psS.tile([128, BW], F32, tag="pss")
                for kd in range(DT):
                    nc.tensor.matmul(
                        ps_s[:],
                        ptt[:, kd, ts(t, 128)],
                        balt[:, kd, t * 128 + 4 : t * 128 + 4 + BW],
                        start=(kd == 0), stop=(kd == DT - 1),
                    )
                # softmax with exp(x) ~= 1+x (|s| <= ~0.04); maskC folds the
                # 1/(K1*L) descale of the raw scores
                sm = smp.tile([128, BW], BF16, tag="sm")
                nc.vector.tensor_tensor(out=sm[:], in0=ps_s[:], in1=maskC[:],
                                        op=AOP.mult)
                rs = smp.tile([128, 1], F32, tag="rs")
                nc.vector.tensor_reduce(out=rs[:], in_=sm[:],
                                        axis=mybir.AxisListType.X, op=AOP.add)
                rs9 = smp.tile([128, 1], F32, tag="rs9")
                nc.vector.tensor_scalar(out=rs9[:], in0=rs[:], scalar1=float(W9),
                                        scalar2=None, op0=AOP.add)
                racc = smp.tile([128, 1], F32, tag="racc")
                nc.vector.reciprocal(racc[:], rs9[:])
                sm1 = smp.tile([128, BW], BF16, tag="sm1")
                nc.vector.tensor_tensor(out=sm1[:], in0=sm[:], in1=maskB[:],
                                        op=AOP.add)
                attnw = smp.tile([128, BW], BF16, tag="attnw")
                nc.vector.tensor_scalar(out=attnw[:], in0=sm1[:], scalar1=racc[:],
                                        scalar2=None, op0=AOP.mult)
                # transpose the attention block into one packed bf16 tile
                # (PSUM start_tensor_calc zeroing is per-address; verified)
                pT = psS.tile([128, 2, 128], BF16, tag="ptt")
                nc.tensor.transpose(pT[:, 0, :], attnw[:, 0:128], ident[:])
                nc.tensor.transpose(pT[0:8, 1, :], attnw[:, 128:BW], ident[:])
                aT1 = atp.tile([128, 128], BF16, tag="at1")
                nc.vector.tensor_copy(aT1[:], pT[:, 0, :])
                aT2 = atp.tile([8, 128], BF16, tag="at2")
                nc.vector.tensor_copy(aT2[:], pT[0:8, 1, :])
                # ctx^T (feature-major) = V_rows^T @ attn^T, in fp8*CT_S;
                # the four dt accumulators pack into one PSUM bank
                ctile = ctp.tile([128, DT, 128], FP8, tag="ct")
                pc = psC.tile([128, DT, 128], F32, tag="pc")
                for dt_ in range(DT):
                    nc.tensor.matmul(pc[:, dt_, :], vrows[:, t, ts(dt_, 128)],
                                     aT1[:], start=True, stop=False)
                    nc.tensor.matmul(pc[:, dt_, :], vrows[0:8, t + 1, ts(dt_, 128)],
                                     aT2[:], start=False, stop=True)
                    nc.scalar.activation(out=ctile[:, dt_, :], in_=pc[:, dt_, :],
                                         func=ACTF.Copy, scale=CT_S)
                # rel = ctx @ Wo^T as fp8 DoubleRow; psr carries CT_S*K2
                psr = psR.tile([128, D], F32, tag="psrel")
                for i in range(DT // 2):
                    nc.tensor.matmul(
                        psr[:], ctile[:, 2 * i : 2 * i + 2, :],
                        wot8All[:, 2 * i : 2 * i + 2, :],
                        start=(i == 0), stop=(i == DT // 2 - 1),
                        perf_mode=DR,
                    )
                apbt = p3c.tile([128, D], F32, tag="apb")
                nc.sync.dma_start(apbt[:], Apb[ts(t, 128), :])
                h = p3c.tile([128, D], F32, tag="h")
                nc.vector.scalar_tensor_tensor(
                    out=h[:], in0=psr[:], scalar=1.0 / (CT_S * K2), in1=apbt[:],
                    op0=AOP.mult, op1=AOP.add,
                )
                st6 = p3c.tile([128, 6], F32, tag="st6")
                nc.vector.bn_stats(out=st6[:], in_=h[:])
                mv = p3c.tile([128, 2], F32, tag="mv")
                nc.vector.bn_aggr(out=mv[:], in_=st6[:])
                sdv = p3c.tile([128, 1], F32, tag="sdv")
                nc.scalar.activation(out=sdv[:], in_=mv[:, 1:2], func=ACTF.Sqrt,
                                     bias=epsS[:], scale=1.0)
                rstd = p3c.tile([128, 1], F32, tag="rstd")
                nc.vector.reciprocal(rstd[:], sdv[:])
                hn = p3c.tile([128, D], F32, tag="hn")
                nc.vector.tensor_scalar(
                    out=hn[:], in0=h[:], scalar1=mv[:, 0:1], scalar2=rstd[:],
                    op0=AOP.subtract, op1=AOP.mult,
                )
                hg = p3c.tile([128, D], F32, tag="hg")
                nc.gpsimd.tensor_tensor(out=hg[:], in0=hn[:], in1=gamB[:],
                                        op=AOP.mult)
                hb = p3c.tile([128, D], F32, tag="hb")
                nc.gpsimd.tensor_tensor(out=hb[:], in0=hg[:], in1=bet2B[:],
                                        op=AOP.add)
                aoutt = p3c.tile([128, D], F32, tag="aout")
                nc.vector.scalar_tensor_tensor(
                    out=aoutt[:], in0=apbt[:], scalar=flagcS[:], in1=hb[:],
                    op0=AOP.mult, op1=AOP.add,
                )
                nc.sync.dma_start(A_out[ts(t, 128), :], aoutt[:])
                nc.scalar.copy(aout8[:, t, :], aoutt[:])

            emit_conv1_chunk(*C1CH[0])
            for i in range(0, 3):
                emit_vrow(i)
            for t in range(0, 2):
                emit_attn(t)
            emit_conv1_chunk(*C1CH[1])
            for i in range(3, 7):
                emit_vrow(i)
            for t in range(2, 6):
                emit_attn(t)
            emit_conv1_chunk(*C1CH[2])
            for i in range(7, 11):
                emit_vrow(i)
            for t in range(6, 10):
                emit_attn(t)
            emit_conv1_chunk(*C1CH[3])
            for i in range(11, 15):
                emit_vrow(i)
            for t in range(10, 14):
                emit_attn(t)
            emit_conv1_chunk(*C1CH[4])
            for i in range(15, 17):
                emit_vrow(i)
            for t in range(14, 16):
                emit_attn(t)

        # ================= conv2: partial B_out (delta part) =============
        NCH = L // D
        with tc.tile_pool(name="outp", bufs=4) as outp, \
             tc.tile_pool(name="ps4", bufs=4, space="PSUM") as ps4:
            for nch in range(NCH):
                for m in range(DT):
                    ps = ps4.tile([128, D], F32, tag="ps4")
                    for k in range(NT // 2):
                        q0 = nch * D - 256 * k - Q2MIN
                        nc.tensor.matmul(
                            ps[:], aout8[:, 2 * k : 2 * k + 2, ts(m, 128)],
                            wr2[:, :, q0 : q0 + D],
                            start=(k == 0), stop=(k == NT // 2 - 1), perf_mode=DR,
                        )
                    osb = outp.tile([128, D], F32, tag="osb")
                    nc.scalar.copy(osb[:], ps[:])
                    nc.sync.dma_start(BT_part[ts(m, 128), ts(nch, D)], osb[:])

    _split_excess_waits(nc)
    return nc


_NC_CACHE = {}


def _get_nc():
    if "nc" not in _NC_CACHE:
        _NC_CACHE["nc"] = _build_nc()
    return _NC_CACHE["nc"]


def _gate_flag(A):
    """Replicate reference _gate on host (fp64; decision margin is ~0.7)."""
    A = np.asarray(A, np.float64)
    pooled = A.mean(axis=1)
    sims = pooled @ pooled.T
    sims = sims - np.eye(sims.shape[0]) * 1e9
    srt = np.sort(sims, axis=-1)
    margin = srt[:, -1] - srt[:, -2]
    m = sims.max(axis=-1, keepdims=True)
    logp = sims - m - np.log(np.exp(sims - m).sum(axis=-1, keepdims=True))
    probs = np.exp(logp)
    entropy = -(probs * np.log(probs + 1e-9)).sum(axis=-1)
    confidence = margin - 0.5 * entropy
    fire = confidence < THRESHOLD
    return bool(fire.any())


def kernel(A, B, phase_weights, Wq, Wk, Wv, Wo, bo, ln_scale, ln_bias):
    A = np.asarray(A, np.float32)
    B = np.asarray(B, np.float32)
    phase_weights = np.asarray(phase_weights, np.float32)
    Wq, Wk, Wv, Wo = (np.asarray(x, np.float32) for x in (Wq, Wk, Wv, Wo))
    bo = np.asarray(bo, np.float32)
    ln_scale = np.asarray(ln_scale, np.float32)
    ln_bias = np.asarray(ln_bias, np.float32)

    nc = _get_nc()

    pw = phase_weights.astype(np.float64)
    wv = np.exp(pw - pw.max())
    wv = wv / wv.sum()                      # softmax weights, fp64
    ubar = 1.0 / L
    delta = wv - ubar
    dmax = max(np.abs(delta).max(), 1e-30)
    SD = 192.0 / dmax                       # scale residual into fp8 range
    d8 = (delta * SD).astype(ml_dtypes.float8_e4m3)

    flag = 1.0 if _gate_flag(A) else 0.0
    flagc = np.float32(1.0 - flag)
    gam16 = (flag * ln_scale).astype(ml_dtypes.bfloat16)

    Wqk8 = ((Wq.T @ Wk) * (K1 / np.sqrt(np.float32(D)))).astype(
        ml_dtypes.float8_e4m3)
    WvT = (Wv.T / L).astype(ml_dtypes.bfloat16)
    WoT8 = (Wo.T * K2).astype(ml_dtypes.float8_e4m3)

    # band masks [128, 136]: row p attends to j in [p, p+8]
    jj = np.arange(BW)[None, :]
    pp = np.arange(128)[:, None]
    maskb = ((jj >= pp) & (jj <= pp + 2 * RADIUS))
    maskB = maskb.astype(ml_dtypes.bfloat16)
    maskC = (maskb / (K1 * L)).astype(ml_dtypes.bfloat16)

    # fp8 generator images of the delta-circulant, one per half
    p_ = np.arange(128)[:, None, None]
    i_ = np.arange(2)[None, :, None]
    wr1s, wr2s = [], []
    for h in range(2):
        own0 = h * HALF
        m1 = np.arange(M1)[None, None, :]
        idx1 = (own0 - HALO + (m1 + Q1MIN) - 128 * i_ - p_) % L
        wr1s.append(d8[idx1])
        m2 = np.arange(M2)[None, None, :]
        idx2 = (own0 + 128 * i_ + p_ - (m2 + Q2MIN)) % L
        wr2s.append(d8[idx2])

    in_maps = []
    for b in range(Bsz):
        # exact rank-1 parts: attention on mean-removed V; the constant
        # vbar @ Wo.T rides along with A + bo (and is cancelled by bet2
        # in the no-fire path so A_out == A stays exact)
        vbar = (B[b].sum(axis=0, dtype=np.float64) / L) @ Wv.T.astype(np.float64)
        rel_const = (vbar @ Wo.T.astype(np.float64)).astype(np.float32)
        bet2_16 = (flag * ln_bias - flagc * (bo + rel_const)).astype(
            ml_dtypes.bfloat16)
        B8 = B[b].astype(ml_dtypes.float8_e4m3)
        for h in range(2):
            own0 = h * HALF
            in_maps.append({
                "Bin8": B8,
                "WR1": wr1s[h],
                "WR2": wr2s[h],
                "AT8": np.ascontiguousarray(A[b, own0 : own0 + HALF].T).astype(
                    ml_dtypes.float8_e4m3
                ),
                "Apb": A[b, own0 : own0 + HALF] + bo + rel_const[None, :],
                "Wqk8": Wqk8,
                "WvT": WvT,
                "WoT8": WoT8,
                "MaskC": maskC,
                "MaskB": maskB,
                "gam": gam16,
                "bet2": bet2_16,
                "flagc": np.array([flagc], np.float32),
                "scal1": np.array([L / SD], np.float32),
            })

    res = run_bass_kernel_spmd(nc, in_maps, core_ids=list(range(8)))

    A_out = np.empty((Bsz, L, D), np.float32)
    B_out = np.empty((Bsz, L, D), np.float32)
    for b in range(Bsz):
        r0 = res.results[2 * b]
        r1 = res.results[2 * b + 1]
        A_out[b, :HALF] = r0["A_out"]
        A_out[b, HALF:] = r1["A_out"]
        # delta part from device; exact rank-1 mean part added on host
        B_out[b] = (r0["BT_part"] + r1["BT_part"]).T / np.float32(SD)
        B_out[b] += A_out[b].sum(axis=0, dtype=np.float64).astype(np.float32)[
            None, :] * np.float32(ubar)
    return A_out, B_out
